# revision 1
# baseline (speedup 1.0000x reference)
"""Trainium2 Bass kernel for KMeans assignment (argmin over centroid distances).

Problem: x [131072, 768] f32, centroids [768, 2000] f32
Output:  argmin_k ||x_n - c_k||^2  -> int32 [131072]

Math: argmin_k(||x||^2 - 2 x.c_k + ||c_k||^2) = argmax_k(x.c_k - 0.5||c_k||^2).
Per-core (data-parallel over 8 cores, 16384 rows each):
  - keep centroids resident in SBUF (bf16 hi/lo split for near-fp32 matmul
    precision at bf16 PE rate: x.c = xh.ch + xh.cl + xl.ch, dropping xl.cl)
  - per 128-row tile: DMA x, cast hi/lo, PE-transpose to [d, n] weights,
    matmul-accumulate scores into PSUM, DVE adds bias (-0.5||c||^2, broadcast
    from host), DVE max/max_index gives argmax along free axis.
"""

import os
import sys

for _p in ("/opt/trn_rl_repo",):
    if _p not in sys.path and os.path.isdir(_p):
        sys.path.insert(0, _p)

from contextlib import ExitStack

import numpy as np

import concourse.bass as bass
import concourse.tile as tile
from concourse import bacc, mybir
from concourse.bass_utils import run_bass_kernel_spmd

try:
    import ml_dtypes

    BF16 = np.dtype(ml_dtypes.bfloat16)
except ImportError:  # pragma: no cover
    BF16 = None

N, D, K = 131072, 768, 2000
NCORES = 8
NSH = N // NCORES  # 16384 rows per core
P = 128
DT = D // P  # 6 contraction tiles
# score chunks, each within one PSUM bank (<=512 fp32)
KOFF = [0, 512, 1024, 1536]
KW = [512, 512, 512, 464]
NB = 4

F32 = mybir.dt.float32
BF = mybir.dt.bfloat16
U32 = mybir.dt.uint32


def build_nc_screen(n_rows: int = NSH):
    """Phase-1 screening program: single bf16 matmul pass.

    Bias (-0.5||c||^2) is folded into the matmul as two extra contraction
    rows (ones-weights x [bias_hi; bias_lo]) so the vector engine only runs
    max/max_index. Outputs the argmax index and the top-2 score values per
    row; rows with a small top-2 margin get recomputed exactly in phase 2.
    """
    assert n_rows % P == 0
    nt = n_rows // P
    nc = bacc.Bacc("TRN2", target_bir_lowering=False, debug=False)

    x = nc.dram_tensor("x", [n_rows, D], F32, kind="ExternalInput").ap()
    c_in = nc.dram_tensor("c", [D, K], BF, kind="ExternalInput").ap()
    bias2 = nc.dram_tensor("bias2", [2, K], BF, kind="ExternalInput").ap()
    ones = nc.dram_tensor("ones", [2, P], BF, kind="ExternalInput").ap()
    ident = nc.dram_tensor("ident", [P, P], BF, kind="ExternalInput").ap()
    out = nc.dram_tensor("out", [n_rows, 1], U32, kind="ExternalOutput").ap()
    vals = nc.dram_tensor("vals", [n_rows, 2], F32, kind="ExternalOutput").ap()

    with tile.TileContext(nc) as tc, ExitStack() as ctx:
        const = ctx.enter_context(tc.tile_pool(name="const", bufs=1))
        xin_p = ctx.enter_context(tc.tile_pool(name="xin", bufs=3))
        xcast_p = ctx.enter_context(tc.tile_pool(name="xcast", bufs=2))
        xtp_p = ctx.enter_context(tc.tile_pool(name="xtp", bufs=1, space="PSUM"))
        xts_p = ctx.enter_context(tc.tile_pool(name="xts", bufs=2))
        sc_p = ctx.enter_context(tc.tile_pool(name="sc", bufs=1, space="PSUM"))
        ss_p = ctx.enter_context(tc.tile_pool(name="ss", bufs=2))
        mx_p = ctx.enter_context(tc.tile_pool(name="mx", bufs=4))

        c3 = c_in.rearrange("(t p) k -> t p k", p=P)
        c_tiles = []
        for d in range(DT):
            ct = const.tile([P, K], BF, tag=f"c_{d}")
            nc.sync.dma_start(ct[:], c3[d])
            c_tiles.append(ct)
        bias_t = const.tile([2, K], BF, tag="bias2")
        nc.sync.dma_start(bias_t[:], bias2[:, :])
        ones_t = const.tile([2, P], BF, tag="ones")
        nc.sync.dma_start(ones_t[:], ones[:, :])
        id_t = const.tile([P, P], BF, tag="ident")
        nc.sync.dma_start(id_t[:], ident[:, :])

        for t in range(nt):
            xin = xin_p.tile([P, D], F32)
            nc.scalar.dma_start(xin[:], x[t * P:(t + 1) * P, :])
            xh = xcast_p.tile([P, D], BF, tag="xh")
            nc.scalar.copy(xh[:], xin[:])
            xtp = xtp_p.tile([P, D], BF)
            for d in range(DT):
                nc.tensor.transpose(
                    xtp[:, d * P:(d + 1) * P], xh[:, d * P:(d + 1) * P], id_t[:])
            xts = xts_p.tile([P, D], BF)
            nc.scalar.copy(xts[:], xtp[:])

            banks = []
            for b in range(NB):
                # first three banks double-buffered: they are what the next
                # tile's first matmuls wait on (PSUM budget: 2+2+2+1 + 1 xtp)
                bank_tile = sc_p.tile([P, KW[b]], F32, tag=f"b{b}",
                                      name=f"bank{b}", bufs=(2 if b < 3 else 1))
                banks.append(bank_tile)
            # bias rows first so each bank's accumulation closes on d == DT-1
            for b in range(NB):
                nc.tensor.matmul(
                    banks[b][:], ones_t[:],
                    bias_t[:, KOFF[b]:KOFF[b] + KW[b]],
                    start=True, stop=False)
            for d in range(DT):
                for b in range(NB):
                    nc.tensor.matmul(
                        banks[b][:], xts[:, d * P:(d + 1) * P],
                        c_tiles[d][:, KOFF[b]:KOFF[b] + KW[b]],
                        start=False, stop=(d == DT - 1))

            ss = ss_p.tile([P, K], F32)
            for b in range(NB):
                nc.scalar.copy(ss[:, KOFF[b]:KOFF[b] + KW[b]], banks[b][:])

            mxv = mx_p.tile([P, 8], F32, tag="mxv")
            nc.vector.max(mxv[:], ss[:])
            mxi = mx_p.tile([P, 8], U32, tag="mxi")
            nc.vector.max_index(mxi[:], mxv[:], ss[:])
            nc.scalar.dma_start(out[t * P:(t + 1) * P, :], mxi[:, 0:1])
            nc.scalar.dma_start(vals[t * P:(t + 1) * P, :], mxv[:, 0:2])

    nc.compile()
    return nc


def build_nc(mode: str = "bf16x3", n_rows: int = NSH):
    """Build + compile the per-core Bass program.

    mode: 'bf16x3' (hi/lo split, 3 bf16 passes), 'fp32', 'fp32r', 'bf16'
    """
    assert n_rows % P == 0
    nt = n_rows // P
    nc = bacc.Bacc("TRN2", target_bir_lowering=False, debug=False)

    x = nc.dram_tensor("x", [n_rows, D], F32, kind="ExternalInput").ap()
    bias = nc.dram_tensor("bias", [P, K], F32, kind="ExternalInput").ap()
    out = nc.dram_tensor("out", [n_rows, 1], U32, kind="ExternalOutput").ap()

    split = mode == "bf16x3"
    cdt = BF if mode in ("bf16x3", "bf16") else F32
    mmdt = {"bf16x3": BF, "bf16": BF, "fp32": F32, "fp32r": mybir.dt.float32r}[mode]

    if split:
        c_hi = nc.dram_tensor("c_hi", [D, K], BF, kind="ExternalInput").ap()
        c_lo = nc.dram_tensor("c_lo", [D, K], BF, kind="ExternalInput").ap()
        c_srcs = [c_hi, c_lo]
    else:
        c_full = nc.dram_tensor("c", [D, K], cdt, kind="ExternalInput").ap()
        c_srcs = [c_full]
    ident = nc.dram_tensor("ident", [P, P], mmdt if mmdt != mybir.dt.float32r else F32,
                           kind="ExternalInput").ap()

    with tile.TileContext(nc) as tc, ExitStack() as ctx:
        const = ctx.enter_context(tc.tile_pool(name="const", bufs=1))
        xin_p = ctx.enter_context(tc.tile_pool(name="xin", bufs=3))
        xcast_p = ctx.enter_context(tc.tile_pool(name="xcast", bufs=2))
        xtp_p = ctx.enter_context(tc.tile_pool(name="xtp", bufs=2, space="PSUM"))
        xts_p = ctx.enter_context(tc.tile_pool(name="xts", bufs=2))
        sc_p = ctx.enter_context(tc.tile_pool(name="sc", bufs=1, space="PSUM"))
        ss_p = ctx.enter_context(tc.tile_pool(name="ss", bufs=2))
        mx_p = ctx.enter_context(tc.tile_pool(name="mx", bufs=4))

        # centroids resident in SBUF: [DT][P, K] per source (hi/lo or single)
        c_tiles = []
        for si, csrc in enumerate(c_srcs):
            c3 = csrc.rearrange("(t p) k -> t p k", p=P)
            tiles = []
            for d in range(DT):
                ct = const.tile([P, K], cdt, tag=f"c{si}_{d}")
                nc.sync.dma_start(ct[:], c3[d])
                tiles.append(ct)
            c_tiles.append(tiles)

        bias_t = const.tile([P, K], F32, tag="bias")
        nc.sync.dma_start(bias_t[:], bias[:, :])
        id_t = const.tile([P, P], ident.dtype, tag="ident")
        nc.sync.dma_start(id_t[:], ident[:, :])

        for t in range(nt):
            xin = xin_p.tile([P, D], F32)
            nc.scalar.dma_start(xin[:], x[t * P:(t + 1) * P, :])

            if split:
                xh = xcast_p.tile([P, D], BF, tag="xh")
                nc.scalar.copy(xh[:], xin[:])
                xl = xcast_p.tile([P, D], BF, tag="xl")
                nc.vector.tensor_sub(xl[:], xin[:], xh[:])
                tsrc = [xh, xl]
            elif mode == "bf16":
                xh = xcast_p.tile([P, D], BF, tag="xh")
                nc.scalar.copy(xh[:], xin[:])
                tsrc = [xh]
            else:
                tsrc = [xin]

            # transpose x tiles -> [d, n] layout for matmul weights
            nsrc = len(tsrc)
            tdt = BF if cdt == BF else F32
            xtp = xtp_p.tile([P, D * nsrc], tdt)
            for si, xsrc in enumerate(tsrc):
                for d in range(DT):
                    nc.tensor.transpose(
                        xtp[:, si * D + d * P: si * D + (d + 1) * P],
                        xsrc[:, d * P:(d + 1) * P],
                        id_t[:],
                    )
            xts = xts_p.tile([P, D * nsrc], tdt)
            nc.scalar.copy(xts[:], xtp[:])

            def w(si, d):
                return xts[:, si * D + d * P: si * D + (d + 1) * P]

            banks = []
            for b in range(NB):
                bank_tile = sc_p.tile([P, KW[b]], F32, tag=f"b{b}", name=f"bank{b}")
                banks.append(bank_tile)
            if split:
                # accumulate xh.ch + xh.cl + xl.ch over d
                for d in range(DT):
                    for b in range(NB):
                        nc.tensor.matmul(
                            banks[b][:], w(0, d),
                            c_tiles[0][d][:, KOFF[b]:KOFF[b] + KW[b]],
                            start=(d == 0), stop=False)
                    for b in range(NB):
                        nc.tensor.matmul(
                            banks[b][:], w(0, d),
                            c_tiles[1][d][:, KOFF[b]:KOFF[b] + KW[b]],
                            start=False, stop=False)
                    for b in range(NB):
                        nc.tensor.matmul(
                            banks[b][:], w(1, d),
                            c_tiles[0][d][:, KOFF[b]:KOFF[b] + KW[b]],
                            start=False, stop=(d == DT - 1))
            else:
                for d in range(DT):
                    for b in range(NB):
                        lhs = w(0, d)
                        rhs = c_tiles[0][d][:, KOFF[b]:KOFF[b] + KW[b]]
                        if mode == "fp32r":
                            lhs = lhs.bitcast(mybir.dt.float32r)
                            rhs = rhs.bitcast(mybir.dt.float32r)
                        nc.tensor.matmul(banks[b][:], lhs, rhs,
                                         start=(d == 0), stop=(d == DT - 1))

            ss = ss_p.tile([P, K], F32)
            for b in range(NB):
                nc.vector.tensor_add(
                    ss[:, KOFF[b]:KOFF[b] + KW[b]], banks[b][:],
                    bias_t[:, KOFF[b]:KOFF[b] + KW[b]])

            mxv = mx_p.tile([P, 8], F32, tag="mxv")
            nc.vector.max(mxv[:], ss[:])
            mxi = mx_p.tile([P, 8], U32, tag="mxi")
            nc.vector.max_index(mxi[:], mxv[:], ss[:])
            nc.scalar.dma_start(out[t * P:(t + 1) * P, :], mxi[:, 0:1])

    nc.compile()
    return nc


def make_in_maps(x: np.ndarray, centroids: np.ndarray, mode: str = "bf16x3",
                 n_rows: int = NSH, n_cores: int = NCORES):
    x = np.ascontiguousarray(x, dtype=np.float32)
    c = np.ascontiguousarray(centroids, dtype=np.float32)
    c_norm = (c.astype(np.float64) ** 2).sum(axis=0)
    bias = np.broadcast_to((-0.5 * c_norm).astype(np.float32), (P, K)).copy()

    base = {"bias": bias}
    if mode == "bf16x3":
        c_hi = c.astype(BF16)
        c_lo = (c - c_hi.astype(np.float32)).astype(BF16)
        base["c_hi"] = c_hi
        base["c_lo"] = c_lo
        base["ident"] = np.eye(P, dtype=BF16)
    elif mode == "bf16":
        base["c"] = c.astype(BF16)
        base["ident"] = np.eye(P, dtype=BF16)
    else:
        base["c"] = c
        base["ident"] = np.eye(P, dtype=np.float32)

    in_maps = []
    for i in range(n_cores):
        m = dict(base)
        m["x"] = x[i * n_rows:(i + 1) * n_rows]
        in_maps.append(m)
    return in_maps


_NC_CACHE = {}
LAST_RESULTS = []  # (label, BassKernelResults) of the most recent kernel() call


def _run_spmd(nc, in_maps, label):
    kw = {}
    if os.environ.get("KMEANS_TRACE"):
        kw["trace"] = True
        kw["tmpdir"] = os.environ.get("KMEANS_TRACE_DIR", "/tmp/km_trace") + "_" + label
        os.makedirs(kw["tmpdir"], exist_ok=True)
    res = run_bass_kernel_spmd(nc, in_maps, core_ids=list(range(NCORES)), **kw)
    LAST_RESULTS.append((label, res))
    return res

# Phase-2 capacity: rows per core recomputed exactly. Margin threshold:
# empirical max bf16 score error on randn data is ~0.2; flag anything under
# 4x that. ~5% of rows get flagged at this threshold.
P2_ROWS = 1024
MARGIN_TH = None  # set below after calibration constant


def _cached_nc(key, builder):
    if key not in _NC_CACHE:
        _NC_CACHE[key] = builder()
    return _NC_CACHE[key]


def make_screen_in_maps(x: np.ndarray, centroids: np.ndarray,
                        n_rows: int = NSH, n_cores: int = NCORES):
    x = np.ascontiguousarray(x, dtype=np.float32)
    c = np.ascontiguousarray(centroids, dtype=np.float32)
    c_norm = (c.astype(np.float64) ** 2).sum(axis=0)
    bias = (-0.5 * c_norm).astype(np.float32)
    bias_hi = bias.astype(BF16)
    bias_lo = (bias - bias_hi.astype(np.float32)).astype(BF16)
    base = {
        "c": c.astype(BF16),
        "bias2": np.stack([bias_hi, bias_lo]),
        "ones": np.ones((2, P), dtype=BF16),
        "ident": np.eye(P, dtype=BF16),
    }
    in_maps = []
    for i in range(n_cores):
        m = dict(base)
        m["x"] = x[i * n_rows:(i + 1) * n_rows]
        in_maps.append(m)
    return in_maps


def _run_exact(x_rows: np.ndarray, centroids: np.ndarray, n_rows: int):
    """Run the exact (bf16x3) program on x_rows padded to n_rows*NCORES."""
    nc = _cached_nc(("bf16x3", n_rows), lambda: build_nc("bf16x3", n_rows))
    total = n_rows * NCORES
    xp = np.zeros((total, D), dtype=np.float32)
    xp[: len(x_rows)] = x_rows
    in_maps = make_in_maps(xp, centroids, mode="bf16x3", n_rows=n_rows)
    res = _run_spmd(nc, in_maps, "phase2")
    out = np.concatenate(
        [res.results[i]["out"].reshape(n_rows) for i in range(NCORES)])
    return out[: len(x_rows)], res


def kernel(x: np.ndarray, centroids: np.ndarray) -> np.ndarray:
    mode = os.environ.get("KMEANS_MODE", "hybrid")
    LAST_RESULTS.clear()
    x = np.asarray(x)
    centroids = np.asarray(centroids)

    if mode != "hybrid":
        nc = _cached_nc((mode, NSH), lambda: build_nc(mode=mode))
        in_maps = make_in_maps(x, centroids, mode=mode)
        res = _run_spmd(nc, in_maps, mode)
        parts = [res.results[i]["out"].reshape(NSH) for i in range(NCORES)]
        return np.concatenate(parts).astype(np.int32)

    # phase 1: bf16 screen with top-2 margins
    nc1 = _cached_nc(("screen", NSH), lambda: build_nc_screen(NSH))
    in_maps = make_screen_in_maps(x, centroids)
    res1 = _run_spmd(nc1, in_maps, "phase1")
    idx = np.concatenate(
        [res1.results[i]["out"].reshape(NSH) for i in range(NCORES)]
    ).astype(np.int32)
    vals = np.concatenate(
        [res1.results[i]["vals"].reshape(NSH, 2) for i in range(NCORES)])

    margin = vals[:, 0] - vals[:, 1]
    th = float(os.environ.get("KMEANS_MARGIN_TH", "0.8"))
    flagged = np.flatnonzero(margin < th)

    # phase 2: exact recompute of flagged rows; pick the smallest padded
    # program that covers the count, chunking in the (unexpected) overflow case
    sizes = [512, 1024, 1536, 2048]
    per_core = min((s for s in sizes if s * NCORES >= len(flagged)),
                   default=sizes[-1])
    cap = per_core * NCORES
    for s in range(0, len(flagged), cap):
        rows = flagged[s:s + cap]
        exact_idx, _ = _run_exact(x[rows], centroids, per_core)
        idx[rows] = exact_idx
    return idx



# revision 7
# speedup vs baseline: 1.4872x; 1.4872x over previous
"""Trainium2 Bass kernel for KMeans assignment (argmin over centroid distances).

Problem: x [131072, 768] f32, centroids [768, 2000] f32
Output:  argmin_k ||x_n - c_k||^2  -> int32 [131072]

Math: argmin_k(||x||^2 - 2 x.c_k + ||c_k||^2) = argmax_k(x.c_k - 0.5||c_k||^2).
Per-core (data-parallel over 8 cores, 16384 rows each):
  - keep centroids resident in SBUF (bf16 hi/lo split for near-fp32 matmul
    precision at bf16 PE rate: x.c = xh.ch + xh.cl + xl.ch, dropping xl.cl)
  - per 128-row tile: DMA x, cast hi/lo, PE-transpose to [d, n] weights,
    matmul-accumulate scores into PSUM, DVE adds bias (-0.5||c||^2, broadcast
    from host), DVE max/max_index gives argmax along free axis.
"""

import os
import sys

for _p in ("/opt/trn_rl_repo",):
    if _p not in sys.path and os.path.isdir(_p):
        sys.path.insert(0, _p)

from contextlib import ExitStack

import numpy as np

import concourse.bass as bass
import concourse.tile as tile
from concourse import bacc, mybir
from concourse.bass_utils import run_bass_kernel_spmd

try:
    import ml_dtypes

    BF16 = np.dtype(ml_dtypes.bfloat16)
except ImportError:  # pragma: no cover
    BF16 = None

N, D, K = 131072, 768, 2000
NCORES = 8
NSH = N // NCORES  # 16384 rows per core
P = 128
DT = D // P  # 6 contraction tiles
# score chunks, each within one PSUM bank (<=512 fp32)
KOFF = [0, 512, 1024, 1536]
KW = [512, 512, 512, 464]
NB = 4

F32 = mybir.dt.float32
BF = mybir.dt.bfloat16
U32 = mybir.dt.uint32


def build_nc_screen(n_rows: int = NSH):
    """Phase-1 screening program: single bf16 matmul pass.

    Bias (-0.5||c||^2) is folded into the matmul as two extra contraction
    rows (ones-weights x [bias_hi; bias_lo]) so the vector engine only runs
    max/max_index. Outputs the argmax index and the top-2 score values per
    row; rows with a small top-2 margin get recomputed exactly in phase 2.
    """
    assert n_rows % P == 0
    nt = n_rows // P
    nc = bacc.Bacc("TRN2", target_bir_lowering=False, debug=False)

    x = nc.dram_tensor("x", [n_rows, D], F32, kind="ExternalInput").ap()
    c_in = nc.dram_tensor("c", [D, K], BF, kind="ExternalInput").ap()
    bias2 = nc.dram_tensor("bias2", [2, K], BF, kind="ExternalInput").ap()
    ones = nc.dram_tensor("ones", [2, P], BF, kind="ExternalInput").ap()
    ident = nc.dram_tensor("ident", [P, P], BF, kind="ExternalInput").ap()
    out = nc.dram_tensor("out", [n_rows, 1], U32, kind="ExternalOutput").ap()
    vals = nc.dram_tensor("vals", [n_rows, 2], F32, kind="ExternalOutput").ap()

    with tile.TileContext(nc) as tc, ExitStack() as ctx:
        const = ctx.enter_context(tc.tile_pool(name="const", bufs=1))
        xin_p = ctx.enter_context(tc.tile_pool(name="xin", bufs=3))
        xcast_p = ctx.enter_context(tc.tile_pool(name="xcast", bufs=2))
        xtp_p = ctx.enter_context(tc.tile_pool(name="xtp", bufs=1, space="PSUM"))
        xts_p = ctx.enter_context(tc.tile_pool(name="xts", bufs=2))
        sc_p = ctx.enter_context(tc.tile_pool(name="sc", bufs=1, space="PSUM"))
        ss_p = ctx.enter_context(tc.tile_pool(name="ss", bufs=2))
        mx_p = ctx.enter_context(tc.tile_pool(name="mx", bufs=4))

        c3 = c_in.rearrange("(t p) k -> t p k", p=P)
        c_tiles = []
        for d in range(DT):
            ct = const.tile([P, K], BF, tag=f"c_{d}")
            nc.sync.dma_start(ct[:], c3[d])
            c_tiles.append(ct)
        bias_t = const.tile([2, K], BF, tag="bias2")
        nc.sync.dma_start(bias_t[:], bias2[:, :])
        ones_t = const.tile([2, P], BF, tag="ones")
        nc.sync.dma_start(ones_t[:], ones[:, :])
        id_t = const.tile([P, P], BF, tag="ident")
        nc.sync.dma_start(id_t[:], ident[:, :])

        for t in range(nt):
            xin = xin_p.tile([P, D], F32)
            nc.scalar.dma_start(xin[:], x[t * P:(t + 1) * P, :])
            xh = xcast_p.tile([P, D], BF, tag="xh")
            nc.scalar.copy(xh[:], xin[:])
            xtp = xtp_p.tile([P, D], BF)
            for d in range(DT):
                nc.tensor.transpose(
                    xtp[:, d * P:(d + 1) * P], xh[:, d * P:(d + 1) * P], id_t[:])
            xts = xts_p.tile([P, D], BF)
            nc.scalar.copy(xts[:], xtp[:])

            banks = []
            for b in range(NB):
                # first three banks double-buffered: they are what the next
                # tile's first matmuls wait on (PSUM budget: 2+2+2+1 + 1 xtp)
                bank_tile = sc_p.tile([P, KW[b]], F32, tag=f"b{b}",
                                      name=f"bank{b}", bufs=(2 if b < 3 else 1))
                banks.append(bank_tile)
            # bias rows first so each bank's accumulation closes on d == DT-1
            for b in range(NB):
                nc.tensor.matmul(
                    banks[b][:], ones_t[:],
                    bias_t[:, KOFF[b]:KOFF[b] + KW[b]],
                    start=True, stop=False)
            for d in range(DT):
                for b in range(NB):
                    nc.tensor.matmul(
                        banks[b][:], xts[:, d * P:(d + 1) * P],
                        c_tiles[d][:, KOFF[b]:KOFF[b] + KW[b]],
                        start=False, stop=(d == DT - 1))

            ss = ss_p.tile([P, K], F32)
            for b in range(NB):
                nc.scalar.copy(ss[:, KOFF[b]:KOFF[b] + KW[b]], banks[b][:])

            mxv = mx_p.tile([P, 8], F32, tag="mxv")
            nc.vector.max(mxv[:], ss[:])
            mxi = mx_p.tile([P, 8], U32, tag="mxi")
            nc.vector.max_index(mxi[:], mxv[:], ss[:])
            nc.scalar.dma_start(out[t * P:(t + 1) * P, :], mxi[:, 0:1])
            nc.scalar.dma_start(vals[t * P:(t + 1) * P, :], mxv[:, 0:2])

    nc.compile()
    return nc


def build_nc_f16(n_rows: int = NSH):
    """fp16 screening program, v2 of the screen.

    Differences from build_nc_screen (bf16):
      - x arrives from the host pre-transposed AND pre-cast to fp16 in tile
        layout xt[t, d, c, col] = x[t*128+col, c*128+d]: no on-device cast,
        no PE transpose, no PSUM->SBUF staging copy. fp16 mm runs at bf16
        speed with ~8x less quantization error -> far fewer phase-2 rows.
      - scores accumulate into ONE 4-bank PSUM tile [128, 2000] so DVE
        max/max_index read the banks directly (no scalar ss copy).
      - outputs (top-8 values f32 + indices u16) collect in SBUF and ship
        as two DMAs at the end: nothing downstream blocks on DVE results,
        so the PE never idles (avoids HAM re-throttle to 1.2 GHz).
      - x DMA issued from the otherwise-idle gpsimd sequencer.
    """
    assert n_rows % P == 0
    nt = n_rows // P
    nc = bacc.Bacc("TRN2", target_bir_lowering=False, debug=False)

    F16 = mybir.dt.float16
    U16 = mybir.dt.uint16

    SHIFT = 384.0  # ~ +0.5*E||c||^2: recenters scores near 0 so fp16 ulp is fine

    xt = nc.dram_tensor("xt", [nt, P, D], F16, kind="ExternalInput").ap()
    c_in = nc.dram_tensor("c", [D, K], F16, kind="ExternalInput").ap()
    bias2 = nc.dram_tensor("bias2", [2, K], F16, kind="ExternalInput").ap()
    ones = nc.dram_tensor("ones", [2, P], F16, kind="ExternalInput").ap()
    shift = nc.dram_tensor("shift", [P, 1], F32, kind="ExternalInput").ap()
    outv = nc.dram_tensor("outv", [P, nt * 8], F16, kind="ExternalOutput").ap()
    outi = nc.dram_tensor("outi", [P, nt * 8], U16, kind="ExternalOutput").ap()

    with tile.TileContext(nc) as tc, ExitStack() as ctx:
        const = ctx.enter_context(tc.tile_pool(name="const", bufs=1))
        xin_p = ctx.enter_context(tc.tile_pool(name="xin", bufs=4))
        sc_p = ctx.enter_context(tc.tile_pool(name="sc", bufs=2, space="PSUM"))
        sf_p = ctx.enter_context(tc.tile_pool(name="sf", bufs=2))
        ob_p = ctx.enter_context(tc.tile_pool(name="ob", bufs=1))

        c3 = c_in.rearrange("(t p) k -> t p k", p=P)
        c_tiles = []
        for d in range(DT):
            ct = const.tile([P, K], F16, tag=f"c_{d}", name=f"c_{d}")
            nc.sync.dma_start(ct[:], c3[d])
            c_tiles.append(ct)
        bias_t = const.tile([2, K], F16, tag="bias2", name="bias2")
        nc.sync.dma_start(bias_t[:], bias2[:, :])
        ones_t = const.tile([2, P], F16, tag="ones", name="ones")
        nc.sync.dma_start(ones_t[:], ones[:, :])
        shift_t = const.tile([P, 1], F32, tag="shift", name="shift")
        nc.sync.dma_start(shift_t[:], shift[:, :])

        outv_t = ob_p.tile([P, nt * 8], F16, tag="outv", name="outv_t")
        outi_t = ob_p.tile([P, nt * 8], U16, tag="outi", name="outi_t")

        for t in range(nt):
            xts = xin_p.tile([P, D], F16, name="xts")
            nc.gpsimd.dma_start(xts[:], xt[t])

            ss = sc_p.tile([P, K], F32, name="ss")
            for b in range(NB):
                nc.tensor.matmul(
                    ss[:, KOFF[b]:KOFF[b] + KW[b]], ones_t[:],
                    bias_t[:, KOFF[b]:KOFF[b] + KW[b]],
                    start=True, stop=False, skip_group_check=True)
            for d in range(DT):
                for b in range(NB):
                    nc.tensor.matmul(
                        ss[:, KOFF[b]:KOFF[b] + KW[b]],
                        xts[:, d * P:(d + 1) * P],
                        c_tiles[d][:, KOFF[b]:KOFF[b] + KW[b]],
                        start=False, stop=(d == DT - 1), skip_group_check=True)

            # shifted fp16 copy: fp16 max/max_index pairs run ~2x faster than
            # f32/PSUM ones (no inter-op pipe-drain bubbles)
            sf = sf_p.tile([P, K], F16, name="sf")
            nc.scalar.activation(sf[:], ss[:],
                                 mybir.ActivationFunctionType.Identity,
                                 bias=shift_t[:], scale=1.0)

            vslot = outv_t[:, t * 8:(t + 1) * 8]
            nc.vector.max(vslot, sf[:])
            nc.vector.max_index(outi_t[:, t * 8:(t + 1) * 8], vslot, sf[:])

        nc.sync.dma_start(outv[:, :], outv_t[:])
        nc.sync.dma_start(outi[:, :], outi_t[:])

    nc.compile()
    return nc


def build_nc_f16x3(n_rows: int):
    """Phase-2 exact rescore, fp16 hi/lo 3-term (error ~3e-6 on randn data).

    Same pre-transposed tile layout as the f16 screen, but x ships as
    [xh; xl] (fp16 value + fp16 residual) and c as ch + cl constants.
    s = xh.ch + xh.cl + xl.ch (+bias), argmax via f32-PSUM max/max_index
    (fp16 reduction would blur the tight margins phase-2 rows have).
    """
    assert n_rows % P == 0
    nt = n_rows // P
    nc = bacc.Bacc("TRN2", target_bir_lowering=False, debug=False)

    F16 = mybir.dt.float16
    U16 = mybir.dt.uint16

    xt = nc.dram_tensor("xt", [nt, P, 2 * D], F16, kind="ExternalInput").ap()
    ch_in = nc.dram_tensor("ch", [D, K], F16, kind="ExternalInput").ap()
    cl_in = nc.dram_tensor("cl", [D, K], F16, kind="ExternalInput").ap()
    bias2 = nc.dram_tensor("bias2", [2, K], F16, kind="ExternalInput").ap()
    ones = nc.dram_tensor("ones", [2, P], F16, kind="ExternalInput").ap()
    outi = nc.dram_tensor("outi", [P, nt * 8], U16, kind="ExternalOutput").ap()

    with tile.TileContext(nc) as tc, ExitStack() as ctx:
        const = ctx.enter_context(tc.tile_pool(name="const", bufs=1))
        xin_p = ctx.enter_context(tc.tile_pool(name="xin", bufs=3))
        sc_p = ctx.enter_context(tc.tile_pool(name="sc", bufs=2, space="PSUM"))
        mx_p = ctx.enter_context(tc.tile_pool(name="mx", bufs=2))
        ob_p = ctx.enter_context(tc.tile_pool(name="ob", bufs=1))

        c_tiles = {}
        for nm, src in (("ch", ch_in), ("cl", cl_in)):
            c3 = src.rearrange("(t p) k -> t p k", p=P)
            for d in range(DT):
                ct = const.tile([P, K], F16, tag=f"{nm}_{d}", name=f"{nm}_{d}")
                nc.sync.dma_start(ct[:], c3[d])
                c_tiles[(nm, d)] = ct
        bias_t = const.tile([2, K], F16, tag="bias2", name="bias2")
        nc.sync.dma_start(bias_t[:], bias2[:, :])
        ones_t = const.tile([2, P], F16, tag="ones", name="ones")
        nc.sync.dma_start(ones_t[:], ones[:, :])

        outi_t = ob_p.tile([P, nt * 8], U16, tag="outi", name="outi_t")

        for t in range(nt):
            xts = xin_p.tile([P, 2 * D], F16, name="xts")
            nc.gpsimd.dma_start(xts[:], xt[t])

            def w(h, d):  # stationary x chunk: h=0 hi, h=1 lo
                off = h * D + d * P
                return xts[:, off:off + P]

            ss = sc_p.tile([P, K], F32, name="ss")
            for b in range(NB):
                nc.tensor.matmul(
                    ss[:, KOFF[b]:KOFF[b] + KW[b]], ones_t[:],
                    bias_t[:, KOFF[b]:KOFF[b] + KW[b]],
                    start=True, stop=False, skip_group_check=True)
            terms = [(0, "ch"), (0, "cl"), (1, "ch")]
            for ti, (h, cn) in enumerate(terms):
                for d in range(DT):
                    last = ti == len(terms) - 1 and d == DT - 1
                    for b in range(NB):
                        nc.tensor.matmul(
                            ss[:, KOFF[b]:KOFF[b] + KW[b]], w(h, d),
                            c_tiles[(cn, d)][:, KOFF[b]:KOFF[b] + KW[b]],
                            start=False, stop=last, skip_group_check=True)

            mxv = mx_p.tile([P, 8], F32, name="mxv")
            nc.vector.max(mxv[:], ss[:])
            nc.vector.max_index(outi_t[:, t * 8:(t + 1) * 8], mxv[:], ss[:])

        nc.sync.dma_start(outi[:, :], outi_t[:])

    nc.compile()
    return nc


def make_f16x3_in_maps(x_rows: np.ndarray, centroids: np.ndarray,
                       n_rows: int, n_cores: int = NCORES):
    """x_rows: gathered flagged rows [m, D] f32, padded to n_rows*n_cores."""
    c = np.ascontiguousarray(centroids, dtype=np.float32)
    nt = n_rows // P
    ch = c.astype(np.float16)
    cl = (c - ch.astype(np.float32)).astype(np.float16)
    c_norm = (c.astype(np.float64) ** 2).sum(axis=0)
    b = (-0.5 * c_norm).astype(np.float32)
    b_hi = b.astype(np.float16)
    b_lo = (b - b_hi.astype(np.float32)).astype(np.float16)
    base = {
        "ch": ch, "cl": cl,
        "bias2": np.stack([b_hi, b_lo]),
        "ones": np.ones((2, P), dtype=np.float16),
    }
    total = n_rows * n_cores
    xp = np.zeros((total, D), dtype=np.float32)
    xp[: len(x_rows)] = x_rows
    xh = xp.astype(np.float16)
    xl = (xp - xh.astype(np.float32)).astype(np.float16)
    in_maps = []
    for i in range(n_cores):
        m = dict(base)
        parts = []
        for arr in (xh, xl):
            ac = arr[i * n_rows:(i + 1) * n_rows]
            parts.append(np.ascontiguousarray(
                ac.reshape(nt, P, DT, P).transpose(0, 3, 2, 1)).reshape(nt, P, D))
        m["xt"] = np.concatenate(parts, axis=2)  # [nt, P, 2D]
        in_maps.append(m)
    return in_maps


def _run_exact_f16(x_rows: np.ndarray, centroids: np.ndarray, n_rows: int):
    nc = _cached_nc(("f16x3", n_rows), lambda: build_nc_f16x3(n_rows))
    in_maps = make_f16x3_in_maps(x_rows, centroids, n_rows)
    res = _run_spmd(nc, in_maps, "phase2")
    nt = n_rows // P
    parts = []
    for i in range(NCORES):
        ii = res.results[i]["outi"].reshape(P, nt, 8)
        parts.append(ii.transpose(1, 0, 2).reshape(n_rows, 8)[:, 0])
    out = np.concatenate(parts).astype(np.int32)
    return out[: len(x_rows)], res


def make_f16_in_maps(x: np.ndarray, centroids: np.ndarray,
                     n_rows: int = NSH, n_cores: int = NCORES):
    x = np.ascontiguousarray(x, dtype=np.float32)
    c = np.ascontiguousarray(centroids, dtype=np.float32)
    nt = n_rows // P
    c_norm = (c.astype(np.float64) ** 2).sum(axis=0)
    b = (-0.5 * c_norm).astype(np.float32)
    b_hi = b.astype(np.float16)
    b_lo = (b - b_hi.astype(np.float32)).astype(np.float16)
    base = {
        "c": c.astype(np.float16),
        "bias2": np.stack([b_hi, b_lo]),
        "ones": np.ones((2, P), dtype=np.float16),
        "shift": np.full((P, 1), 384.0, dtype=np.float32),
    }
    x16 = x.astype(np.float16)
    in_maps = []
    for i in range(n_cores):
        xc = x16[i * n_rows:(i + 1) * n_rows]
        # xt[t, d, c, col] = x[t*128+col, c*128+d]
        xt = np.ascontiguousarray(
            xc.reshape(nt, P, DT, P).transpose(0, 3, 2, 1)).reshape(nt, P, D)
        m = dict(base)
        m["xt"] = xt
        in_maps.append(m)
    return in_maps


def build_nc(mode: str = "bf16x3", n_rows: int = NSH):
    """Build + compile the per-core Bass program.

    mode: 'bf16x3' (hi/lo split, 3 bf16 passes), 'fp32', 'fp32r', 'bf16'
    """
    assert n_rows % P == 0
    nt = n_rows // P
    nc = bacc.Bacc("TRN2", target_bir_lowering=False, debug=False)

    x = nc.dram_tensor("x", [n_rows, D], F32, kind="ExternalInput").ap()
    bias = nc.dram_tensor("bias", [P, K], F32, kind="ExternalInput").ap()
    out = nc.dram_tensor("out", [n_rows, 1], U32, kind="ExternalOutput").ap()

    split = mode == "bf16x3"
    cdt = BF if mode in ("bf16x3", "bf16") else F32
    mmdt = {"bf16x3": BF, "bf16": BF, "fp32": F32, "fp32r": mybir.dt.float32r}[mode]

    if split:
        c_hi = nc.dram_tensor("c_hi", [D, K], BF, kind="ExternalInput").ap()
        c_lo = nc.dram_tensor("c_lo", [D, K], BF, kind="ExternalInput").ap()
        c_srcs = [c_hi, c_lo]
    else:
        c_full = nc.dram_tensor("c", [D, K], cdt, kind="ExternalInput").ap()
        c_srcs = [c_full]
    ident = nc.dram_tensor("ident", [P, P], mmdt if mmdt != mybir.dt.float32r else F32,
                           kind="ExternalInput").ap()

    with tile.TileContext(nc) as tc, ExitStack() as ctx:
        const = ctx.enter_context(tc.tile_pool(name="const", bufs=1))
        xin_p = ctx.enter_context(tc.tile_pool(name="xin", bufs=3))
        xcast_p = ctx.enter_context(tc.tile_pool(name="xcast", bufs=2))
        xtp_p = ctx.enter_context(tc.tile_pool(name="xtp", bufs=2, space="PSUM"))
        xts_p = ctx.enter_context(tc.tile_pool(name="xts", bufs=2))
        sc_p = ctx.enter_context(tc.tile_pool(name="sc", bufs=1, space="PSUM"))
        ss_p = ctx.enter_context(tc.tile_pool(name="ss", bufs=2))
        mx_p = ctx.enter_context(tc.tile_pool(name="mx", bufs=4))

        # centroids resident in SBUF: [DT][P, K] per source (hi/lo or single)
        c_tiles = []
        for si, csrc in enumerate(c_srcs):
            c3 = csrc.rearrange("(t p) k -> t p k", p=P)
            tiles = []
            for d in range(DT):
                ct = const.tile([P, K], cdt, tag=f"c{si}_{d}")
                nc.sync.dma_start(ct[:], c3[d])
                tiles.append(ct)
            c_tiles.append(tiles)

        bias_t = const.tile([P, K], F32, tag="bias")
        nc.sync.dma_start(bias_t[:], bias[:, :])
        id_t = const.tile([P, P], ident.dtype, tag="ident")
        nc.sync.dma_start(id_t[:], ident[:, :])

        for t in range(nt):
            xin = xin_p.tile([P, D], F32)
            nc.scalar.dma_start(xin[:], x[t * P:(t + 1) * P, :])

            if split:
                xh = xcast_p.tile([P, D], BF, tag="xh")
                nc.scalar.copy(xh[:], xin[:])
                xl = xcast_p.tile([P, D], BF, tag="xl")
                nc.vector.tensor_sub(xl[:], xin[:], xh[:])
                tsrc = [xh, xl]
            elif mode == "bf16":
                xh = xcast_p.tile([P, D], BF, tag="xh")
                nc.scalar.copy(xh[:], xin[:])
                tsrc = [xh]
            else:
                tsrc = [xin]

            # transpose x tiles -> [d, n] layout for matmul weights
            nsrc = len(tsrc)
            tdt = BF if cdt == BF else F32
            xtp = xtp_p.tile([P, D * nsrc], tdt)
            for si, xsrc in enumerate(tsrc):
                for d in range(DT):
                    nc.tensor.transpose(
                        xtp[:, si * D + d * P: si * D + (d + 1) * P],
                        xsrc[:, d * P:(d + 1) * P],
                        id_t[:],
                    )
            xts = xts_p.tile([P, D * nsrc], tdt)
            nc.scalar.copy(xts[:], xtp[:])

            def w(si, d):
                return xts[:, si * D + d * P: si * D + (d + 1) * P]

            banks = []
            for b in range(NB):
                bank_tile = sc_p.tile([P, KW[b]], F32, tag=f"b{b}", name=f"bank{b}")
                banks.append(bank_tile)
            if split:
                # accumulate xh.ch + xh.cl + xl.ch over d
                for d in range(DT):
                    for b in range(NB):
                        nc.tensor.matmul(
                            banks[b][:], w(0, d),
                            c_tiles[0][d][:, KOFF[b]:KOFF[b] + KW[b]],
                            start=(d == 0), stop=False)
                    for b in range(NB):
                        nc.tensor.matmul(
                            banks[b][:], w(0, d),
                            c_tiles[1][d][:, KOFF[b]:KOFF[b] + KW[b]],
                            start=False, stop=False)
                    for b in range(NB):
                        nc.tensor.matmul(
                            banks[b][:], w(1, d),
                            c_tiles[0][d][:, KOFF[b]:KOFF[b] + KW[b]],
                            start=False, stop=(d == DT - 1))
            else:
                for d in range(DT):
                    for b in range(NB):
                        lhs = w(0, d)
                        rhs = c_tiles[0][d][:, KOFF[b]:KOFF[b] + KW[b]]
                        if mode == "fp32r":
                            lhs = lhs.bitcast(mybir.dt.float32r)
                            rhs = rhs.bitcast(mybir.dt.float32r)
                        nc.tensor.matmul(banks[b][:], lhs, rhs,
                                         start=(d == 0), stop=(d == DT - 1))

            ss = ss_p.tile([P, K], F32)
            for b in range(NB):
                nc.vector.tensor_add(
                    ss[:, KOFF[b]:KOFF[b] + KW[b]], banks[b][:],
                    bias_t[:, KOFF[b]:KOFF[b] + KW[b]])

            mxv = mx_p.tile([P, 8], F32, tag="mxv")
            nc.vector.max(mxv[:], ss[:])
            mxi = mx_p.tile([P, 8], U32, tag="mxi")
            nc.vector.max_index(mxi[:], mxv[:], ss[:])
            nc.scalar.dma_start(out[t * P:(t + 1) * P, :], mxi[:, 0:1])

    nc.compile()
    return nc


def make_in_maps(x: np.ndarray, centroids: np.ndarray, mode: str = "bf16x3",
                 n_rows: int = NSH, n_cores: int = NCORES):
    x = np.ascontiguousarray(x, dtype=np.float32)
    c = np.ascontiguousarray(centroids, dtype=np.float32)
    c_norm = (c.astype(np.float64) ** 2).sum(axis=0)
    bias = np.broadcast_to((-0.5 * c_norm).astype(np.float32), (P, K)).copy()

    base = {"bias": bias}
    if mode == "bf16x3":
        c_hi = c.astype(BF16)
        c_lo = (c - c_hi.astype(np.float32)).astype(BF16)
        base["c_hi"] = c_hi
        base["c_lo"] = c_lo
        base["ident"] = np.eye(P, dtype=BF16)
    elif mode == "bf16":
        base["c"] = c.astype(BF16)
        base["ident"] = np.eye(P, dtype=BF16)
    else:
        base["c"] = c
        base["ident"] = np.eye(P, dtype=np.float32)

    in_maps = []
    for i in range(n_cores):
        m = dict(base)
        m["x"] = x[i * n_rows:(i + 1) * n_rows]
        in_maps.append(m)
    return in_maps


_NC_CACHE = {}
LAST_RESULTS = []  # (label, BassKernelResults) of the most recent kernel() call


def _run_spmd(nc, in_maps, label):
    kw = {}
    if os.environ.get("KMEANS_TRACE"):
        kw["trace"] = True
        kw["tmpdir"] = os.environ.get("KMEANS_TRACE_DIR", "/tmp/km_trace") + "_" + label
        os.makedirs(kw["tmpdir"], exist_ok=True)
    res = run_bass_kernel_spmd(nc, in_maps, core_ids=list(range(NCORES)), **kw)
    LAST_RESULTS.append((label, res))
    return res

# Phase-2 capacity: rows per core recomputed exactly. Margin threshold:
# empirical max bf16 score error on randn data is ~0.2; flag anything under
# 4x that. ~5% of rows get flagged at this threshold.
P2_ROWS = 1024
MARGIN_TH = None  # set below after calibration constant


def _cached_nc(key, builder):
    if key not in _NC_CACHE:
        _NC_CACHE[key] = builder()
    return _NC_CACHE[key]


def make_screen_in_maps(x: np.ndarray, centroids: np.ndarray,
                        n_rows: int = NSH, n_cores: int = NCORES):
    x = np.ascontiguousarray(x, dtype=np.float32)
    c = np.ascontiguousarray(centroids, dtype=np.float32)
    c_norm = (c.astype(np.float64) ** 2).sum(axis=0)
    bias = (-0.5 * c_norm).astype(np.float32)
    bias_hi = bias.astype(BF16)
    bias_lo = (bias - bias_hi.astype(np.float32)).astype(BF16)
    base = {
        "c": c.astype(BF16),
        "bias2": np.stack([bias_hi, bias_lo]),
        "ones": np.ones((2, P), dtype=BF16),
        "ident": np.eye(P, dtype=BF16),
    }
    in_maps = []
    for i in range(n_cores):
        m = dict(base)
        m["x"] = x[i * n_rows:(i + 1) * n_rows]
        in_maps.append(m)
    return in_maps


def _run_exact(x_rows: np.ndarray, centroids: np.ndarray, n_rows: int):
    """Run the exact (bf16x3) program on x_rows padded to n_rows*NCORES."""
    nc = _cached_nc(("bf16x3", n_rows), lambda: build_nc("bf16x3", n_rows))
    total = n_rows * NCORES
    xp = np.zeros((total, D), dtype=np.float32)
    xp[: len(x_rows)] = x_rows
    in_maps = make_in_maps(xp, centroids, mode="bf16x3", n_rows=n_rows)
    res = _run_spmd(nc, in_maps, "phase2")
    out = np.concatenate(
        [res.results[i]["out"].reshape(n_rows) for i in range(NCORES)])
    return out[: len(x_rows)], res


def kernel(x: np.ndarray, centroids: np.ndarray) -> np.ndarray:
    mode = os.environ.get("KMEANS_MODE", "f16")
    LAST_RESULTS.clear()
    x = np.asarray(x)
    centroids = np.asarray(centroids)

    if mode == "f16":
        # phase 1: fp16 screen with top-8 values + indices
        nc1 = _cached_nc(("f16", NSH), lambda: build_nc_f16(NSH))
        in_maps = make_f16_in_maps(x, centroids)
        res1 = _run_spmd(nc1, in_maps, "phase1")
        nt = NSH // P
        idx_parts, val_parts = [], []
        for i in range(NCORES):
            vi = res1.results[i]["outv"].reshape(P, nt, 8).astype(np.float32)
            ii = res1.results[i]["outi"].reshape(P, nt, 8)
            # row n = t*128 + p  ->  [t, p]
            val_parts.append(vi.transpose(1, 0, 2).reshape(NSH, 8))
            idx_parts.append(ii.transpose(1, 0, 2).reshape(NSH, 8))
        vals = np.concatenate(val_parts)
        idx = np.concatenate(idx_parts)[:, 0].astype(np.int32)

        # fp16 values: quantization ~0.06-0.125 near |s'|<=256 plus fp16
        # matmul error ~0.05 -> th=0.3 flags everything at risk
        margin = vals[:, 0] - vals[:, 1]
        th = float(os.environ.get("KMEANS_MARGIN_TH", "0.3"))
        flagged = np.flatnonzero(margin < th)

        sizes = [128, 256, 512, 1024, 2048]
        per_core = min((s for s in sizes if s * NCORES >= len(flagged)),
                       default=sizes[-1])
        cap = per_core * NCORES
        for s in range(0, len(flagged), cap):
            rows = flagged[s:s + cap]
            exact_idx, _ = _run_exact(x[rows], centroids, per_core)
            idx[rows] = exact_idx
        return idx

    if mode != "hybrid":
        nc = _cached_nc((mode, NSH), lambda: build_nc(mode=mode))
        in_maps = make_in_maps(x, centroids, mode=mode)
        res = _run_spmd(nc, in_maps, mode)
        parts = [res.results[i]["out"].reshape(NSH) for i in range(NCORES)]
        return np.concatenate(parts).astype(np.int32)

    # phase 1: bf16 screen with top-2 margins
    nc1 = _cached_nc(("screen", NSH), lambda: build_nc_screen(NSH))
    in_maps = make_screen_in_maps(x, centroids)
    res1 = _run_spmd(nc1, in_maps, "phase1")
    idx = np.concatenate(
        [res1.results[i]["out"].reshape(NSH) for i in range(NCORES)]
    ).astype(np.int32)
    vals = np.concatenate(
        [res1.results[i]["vals"].reshape(NSH, 2) for i in range(NCORES)])

    margin = vals[:, 0] - vals[:, 1]
    th = float(os.environ.get("KMEANS_MARGIN_TH", "0.8"))
    flagged = np.flatnonzero(margin < th)

    # phase 2: exact recompute of flagged rows; pick the smallest padded
    # program that covers the count, chunking in the (unexpected) overflow case
    sizes = [512, 1024, 1536, 2048]
    per_core = min((s for s in sizes if s * NCORES >= len(flagged)),
                   default=sizes[-1])
    cap = per_core * NCORES
    for s in range(0, len(flagged), cap):
        rows = flagged[s:s + cap]
        exact_idx, _ = _run_exact(x[rows], centroids, per_core)
        idx[rows] = exact_idx
    return idx



# revision 10
# speedup vs baseline: 1.6456x; 1.1065x over previous
"""Trainium2 Bass kernel for KMeans assignment (argmin over centroid distances).

Problem: x [131072, 768] f32, centroids [768, 2000] f32
Output:  argmin_k ||x_n - c_k||^2  -> int32 [131072]

Math: argmin_k(||x||^2 - 2 x.c_k + ||c_k||^2) = argmax_k(x.c_k - 0.5||c_k||^2).
Per-core (data-parallel over 8 cores, 16384 rows each):
  - keep centroids resident in SBUF (bf16 hi/lo split for near-fp32 matmul
    precision at bf16 PE rate: x.c = xh.ch + xh.cl + xl.ch, dropping xl.cl)
  - per 128-row tile: DMA x, cast hi/lo, PE-transpose to [d, n] weights,
    matmul-accumulate scores into PSUM, DVE adds bias (-0.5||c||^2, broadcast
    from host), DVE max/max_index gives argmax along free axis.
"""

import os
import sys

for _p in ("/opt/trn_rl_repo",):
    if _p not in sys.path and os.path.isdir(_p):
        sys.path.insert(0, _p)

from contextlib import ExitStack

import numpy as np

import concourse.bass as bass
import concourse.tile as tile
from concourse import bacc, mybir
from concourse.bass_utils import run_bass_kernel_spmd

try:
    import ml_dtypes

    BF16 = np.dtype(ml_dtypes.bfloat16)
except ImportError:  # pragma: no cover
    BF16 = None

N, D, K = 131072, 768, 2000
NCORES = 8
NSH = N // NCORES  # 16384 rows per core
P = 128
DT = D // P  # 6 contraction tiles
# score chunks, each within one PSUM bank (<=512 fp32)
KOFF = [0, 512, 1024, 1536]
KW = [512, 512, 512, 464]
NB = 4

F32 = mybir.dt.float32
BF = mybir.dt.bfloat16
U32 = mybir.dt.uint32


def build_nc_screen(n_rows: int = NSH):
    """Phase-1 screening program: single bf16 matmul pass.

    Bias (-0.5||c||^2) is folded into the matmul as two extra contraction
    rows (ones-weights x [bias_hi; bias_lo]) so the vector engine only runs
    max/max_index. Outputs the argmax index and the top-2 score values per
    row; rows with a small top-2 margin get recomputed exactly in phase 2.
    """
    assert n_rows % P == 0
    nt = n_rows // P
    nc = bacc.Bacc("TRN2", target_bir_lowering=False, debug=False)

    x = nc.dram_tensor("x", [n_rows, D], F32, kind="ExternalInput").ap()
    c_in = nc.dram_tensor("c", [D, K], BF, kind="ExternalInput").ap()
    bias2 = nc.dram_tensor("bias2", [2, K], BF, kind="ExternalInput").ap()
    ones = nc.dram_tensor("ones", [2, P], BF, kind="ExternalInput").ap()
    ident = nc.dram_tensor("ident", [P, P], BF, kind="ExternalInput").ap()
    out = nc.dram_tensor("out", [n_rows, 1], U32, kind="ExternalOutput").ap()
    vals = nc.dram_tensor("vals", [n_rows, 2], F32, kind="ExternalOutput").ap()

    with tile.TileContext(nc) as tc, ExitStack() as ctx:
        const = ctx.enter_context(tc.tile_pool(name="const", bufs=1))
        xin_p = ctx.enter_context(tc.tile_pool(name="xin", bufs=3))
        xcast_p = ctx.enter_context(tc.tile_pool(name="xcast", bufs=2))
        xtp_p = ctx.enter_context(tc.tile_pool(name="xtp", bufs=1, space="PSUM"))
        xts_p = ctx.enter_context(tc.tile_pool(name="xts", bufs=2))
        sc_p = ctx.enter_context(tc.tile_pool(name="sc", bufs=1, space="PSUM"))
        ss_p = ctx.enter_context(tc.tile_pool(name="ss", bufs=2))
        mx_p = ctx.enter_context(tc.tile_pool(name="mx", bufs=4))

        c3 = c_in.rearrange("(t p) k -> t p k", p=P)
        c_tiles = []
        for d in range(DT):
            ct = const.tile([P, K], BF, tag=f"c_{d}")
            nc.sync.dma_start(ct[:], c3[d])
            c_tiles.append(ct)
        bias_t = const.tile([2, K], BF, tag="bias2")
        nc.sync.dma_start(bias_t[:], bias2[:, :])
        ones_t = const.tile([2, P], BF, tag="ones")
        nc.sync.dma_start(ones_t[:], ones[:, :])
        id_t = const.tile([P, P], BF, tag="ident")
        nc.sync.dma_start(id_t[:], ident[:, :])

        for t in range(nt):
            xin = xin_p.tile([P, D], F32)
            nc.scalar.dma_start(xin[:], x[t * P:(t + 1) * P, :])
            xh = xcast_p.tile([P, D], BF, tag="xh")
            nc.scalar.copy(xh[:], xin[:])
            xtp = xtp_p.tile([P, D], BF)
            for d in range(DT):
                nc.tensor.transpose(
                    xtp[:, d * P:(d + 1) * P], xh[:, d * P:(d + 1) * P], id_t[:])
            xts = xts_p.tile([P, D], BF)
            nc.scalar.copy(xts[:], xtp[:])

            banks = []
            for b in range(NB):
                # first three banks double-buffered: they are what the next
                # tile's first matmuls wait on (PSUM budget: 2+2+2+1 + 1 xtp)
                bank_tile = sc_p.tile([P, KW[b]], F32, tag=f"b{b}",
                                      name=f"bank{b}", bufs=(2 if b < 3 else 1))
                banks.append(bank_tile)
            # bias rows first so each bank's accumulation closes on d == DT-1
            for b in range(NB):
                nc.tensor.matmul(
                    banks[b][:], ones_t[:],
                    bias_t[:, KOFF[b]:KOFF[b] + KW[b]],
                    start=True, stop=False)
            for d in range(DT):
                for b in range(NB):
                    nc.tensor.matmul(
                        banks[b][:], xts[:, d * P:(d + 1) * P],
                        c_tiles[d][:, KOFF[b]:KOFF[b] + KW[b]],
                        start=False, stop=(d == DT - 1))

            ss = ss_p.tile([P, K], F32)
            for b in range(NB):
                nc.scalar.copy(ss[:, KOFF[b]:KOFF[b] + KW[b]], banks[b][:])

            mxv = mx_p.tile([P, 8], F32, tag="mxv")
            nc.vector.max(mxv[:], ss[:])
            mxi = mx_p.tile([P, 8], U32, tag="mxi")
            nc.vector.max_index(mxi[:], mxv[:], ss[:])
            nc.scalar.dma_start(out[t * P:(t + 1) * P, :], mxi[:, 0:1])
            nc.scalar.dma_start(vals[t * P:(t + 1) * P, :], mxv[:, 0:2])

    nc.compile()
    return nc


def build_nc_f16(n_rows: int = NSH):
    """fp16 screening program, v2 of the screen.

    Differences from build_nc_screen (bf16):
      - x arrives from the host pre-transposed AND pre-cast to fp16 in tile
        layout xt[t, d, c, col] = x[t*128+col, c*128+d]: no on-device cast,
        no PE transpose, no PSUM->SBUF staging copy. fp16 mm runs at bf16
        speed with ~8x less quantization error -> far fewer phase-2 rows.
      - scores accumulate into ONE 4-bank PSUM tile [128, 2000] so DVE
        max/max_index read the banks directly (no scalar ss copy).
      - outputs (top-8 values f32 + indices u16) collect in SBUF and ship
        as two DMAs at the end: nothing downstream blocks on DVE results,
        so the PE never idles (avoids HAM re-throttle to 1.2 GHz).
      - x DMA issued from the otherwise-idle gpsimd sequencer.
    """
    assert n_rows % P == 0
    nt = n_rows // P
    nc = bacc.Bacc("TRN2", target_bir_lowering=False, debug=False)

    F16 = mybir.dt.float16
    U16 = mybir.dt.uint16

    # bias2/biasr hold hi/lo resp. fp16 of (b + 384): the +384 recenters
    # scores near 0 so every fp16 intermediate keeps small ulp
    xt = nc.dram_tensor("xt", [nt, P, D], F16, kind="ExternalInput").ap()
    c_in = nc.dram_tensor("c", [D, K], F16, kind="ExternalInput").ap()
    bias2 = nc.dram_tensor("bias2", [2, K], F16, kind="ExternalInput").ap()
    biasr = nc.dram_tensor("biasr", [P, K], F16, kind="ExternalInput").ap()
    ones = nc.dram_tensor("ones", [2, P], F16, kind="ExternalInput").ap()
    outv = nc.dram_tensor("outv", [P, nt * 8], F16, kind="ExternalOutput").ap()
    outi = nc.dram_tensor("outi", [P, nt * 8], U16, kind="ExternalOutput").ap()

    with tile.TileContext(nc) as tc, ExitStack() as ctx:
        const = ctx.enter_context(tc.tile_pool(name="const", bufs=1))
        xin_p = ctx.enter_context(tc.tile_pool(name="xin", bufs=4))
        sc_p = ctx.enter_context(tc.tile_pool(name="sc", bufs=2, space="PSUM"))
        sf_p = ctx.enter_context(tc.tile_pool(name="sf", bufs=2))
        ob_p = ctx.enter_context(tc.tile_pool(name="ob", bufs=1))

        # small consts first so tile 0's bias matmul can start immediately
        bias_t = const.tile([2, K], F16, tag="bias2", name="bias2")
        nc.sync.dma_start(bias_t[:], bias2[:, :])
        ones_t = const.tile([2, P], F16, tag="ones", name="ones")
        nc.sync.dma_start(ones_t[:], ones[:, :])
        biasr_t = const.tile([P, K], F16, tag="biasr", name="biasr")
        nc.sync.dma_start(biasr_t[:], biasr[:, :])
        c3 = c_in.rearrange("(t p) k -> t p k", p=P)
        c_tiles = []
        for d in range(DT):
            ct = const.tile([P, K], F16, tag=f"c_{d}", name=f"c_{d}")
            nc.sync.dma_start(ct[:], c3[d])
            c_tiles.append(ct)

        outv_t = ob_p.tile([P, nt * 8], F16, tag="outv", name="outv_t")
        outi_t = ob_p.tile([P, nt * 8], U16, tag="outi", name="outi_t")

        for t in range(nt):
            xts = xin_p.tile([P, D], F16, name="xts")
            nc.gpsimd.dma_start(xts[:], xt[t])

            ss = sc_p.tile([P, K], F32, name="ss")
            # bias via PE ones-matmul for bank 0 only; banks 1-3 get the
            # (b+384) residual added by the DVE on the fp16 copy (2x rate),
            # balancing PE (~5.43us) against DVE (~5.5us) per tile
            nc.tensor.matmul(
                ss[:, 0:KW[0]], ones_t[:], bias_t[:, 0:KW[0]],
                start=True, stop=False, skip_group_check=True)
            for d in range(DT):
                for b in range(NB):
                    nc.tensor.matmul(
                        ss[:, KOFF[b]:KOFF[b] + KW[b]],
                        xts[:, d * P:(d + 1) * P],
                        c_tiles[d][:, KOFF[b]:KOFF[b] + KW[b]],
                        start=(d == 0 and b > 0), stop=(d == DT - 1),
                        skip_group_check=True)

            # fp16 copy: fp16 max/max_index pairs run ~2x faster than
            # f32/PSUM ones (no inter-op pipe-drain bubbles)
            sf = sf_p.tile([P, K], F16, name="sf")
            nc.scalar.copy(sf[:], ss[:])
            nc.vector.tensor_add(sf[:, KW[0]:K], sf[:, KW[0]:K],
                                 biasr_t[:, KW[0]:K])

            vslot = outv_t[:, t * 8:(t + 1) * 8]
            nc.vector.max(vslot, sf[:])
            nc.vector.max_index(outi_t[:, t * 8:(t + 1) * 8], vslot, sf[:])

        nc.sync.dma_start(outv[:, :], outv_t[:])
        nc.sync.dma_start(outi[:, :], outi_t[:])

    nc.compile()
    return nc


def build_nc_f16x3(n_rows: int):
    """Phase-2 exact rescore, fp16 hi/lo 3-term (error ~3e-6 on randn data).

    Same pre-transposed tile layout as the f16 screen, but x ships as
    [xh; xl] (fp16 value + fp16 residual) and c as ch + cl constants.
    s = xh.ch + xh.cl + xl.ch (+bias), argmax via f32-PSUM max/max_index
    (fp16 reduction would blur the tight margins phase-2 rows have).
    """
    assert n_rows % P == 0
    nt = n_rows // P
    nc = bacc.Bacc("TRN2", target_bir_lowering=False, debug=False)

    F16 = mybir.dt.float16
    U16 = mybir.dt.uint16

    xt = nc.dram_tensor("xt", [nt, P, 2 * D], F16, kind="ExternalInput").ap()
    ch_in = nc.dram_tensor("ch", [D, K], F16, kind="ExternalInput").ap()
    cl_in = nc.dram_tensor("cl", [D, K], F16, kind="ExternalInput").ap()
    bias2 = nc.dram_tensor("bias2", [2, K], F16, kind="ExternalInput").ap()
    ones = nc.dram_tensor("ones", [2, P], F16, kind="ExternalInput").ap()
    outi = nc.dram_tensor("outi", [P, nt * 8], U16, kind="ExternalOutput").ap()

    with tile.TileContext(nc) as tc, ExitStack() as ctx:
        const = ctx.enter_context(tc.tile_pool(name="const", bufs=1))
        xin_p = ctx.enter_context(tc.tile_pool(name="xin", bufs=3))
        sc_p = ctx.enter_context(tc.tile_pool(name="sc", bufs=2, space="PSUM"))
        mx_p = ctx.enter_context(tc.tile_pool(name="mx", bufs=2))
        ob_p = ctx.enter_context(tc.tile_pool(name="ob", bufs=1))

        c_tiles = {}
        for nm, src in (("ch", ch_in), ("cl", cl_in)):
            c3 = src.rearrange("(t p) k -> t p k", p=P)
            for d in range(DT):
                ct = const.tile([P, K], F16, tag=f"{nm}_{d}", name=f"{nm}_{d}")
                nc.sync.dma_start(ct[:], c3[d])
                c_tiles[(nm, d)] = ct
        bias_t = const.tile([2, K], F16, tag="bias2", name="bias2")
        nc.sync.dma_start(bias_t[:], bias2[:, :])
        ones_t = const.tile([2, P], F16, tag="ones", name="ones")
        nc.sync.dma_start(ones_t[:], ones[:, :])

        outi_t = ob_p.tile([P, nt * 8], U16, tag="outi", name="outi_t")

        for t in range(nt):
            xts = xin_p.tile([P, 2 * D], F16, name="xts")
            nc.gpsimd.dma_start(xts[:], xt[t])

            def w(h, d):  # stationary x chunk: h=0 hi, h=1 lo
                off = h * D + d * P
                return xts[:, off:off + P]

            ss = sc_p.tile([P, K], F32, name="ss")
            for b in range(NB):
                nc.tensor.matmul(
                    ss[:, KOFF[b]:KOFF[b] + KW[b]], ones_t[:],
                    bias_t[:, KOFF[b]:KOFF[b] + KW[b]],
                    start=True, stop=False, skip_group_check=True)
            terms = [(0, "ch"), (0, "cl"), (1, "ch")]
            for ti, (h, cn) in enumerate(terms):
                for d in range(DT):
                    last = ti == len(terms) - 1 and d == DT - 1
                    for b in range(NB):
                        nc.tensor.matmul(
                            ss[:, KOFF[b]:KOFF[b] + KW[b]], w(h, d),
                            c_tiles[(cn, d)][:, KOFF[b]:KOFF[b] + KW[b]],
                            start=False, stop=last, skip_group_check=True)

            mxv = mx_p.tile([P, 8], F32, name="mxv")
            nc.vector.max(mxv[:], ss[:])
            nc.vector.max_index(outi_t[:, t * 8:(t + 1) * 8], mxv[:], ss[:])

        nc.sync.dma_start(outi[:, :], outi_t[:])

    nc.compile()
    return nc


def make_f16x3_in_maps(x_rows: np.ndarray, centroids: np.ndarray,
                       n_rows: int, n_cores: int = NCORES):
    """x_rows: gathered flagged rows [m, D] f32, padded to n_rows*n_cores."""
    c = np.ascontiguousarray(centroids, dtype=np.float32)
    nt = n_rows // P
    ch = c.astype(np.float16)
    cl = (c - ch.astype(np.float32)).astype(np.float16)
    c_norm = (c.astype(np.float64) ** 2).sum(axis=0)
    b = (-0.5 * c_norm).astype(np.float32)
    b_hi = b.astype(np.float16)
    b_lo = (b - b_hi.astype(np.float32)).astype(np.float16)
    base = {
        "ch": ch, "cl": cl,
        "bias2": np.stack([b_hi, b_lo]),
        "ones": np.ones((2, P), dtype=np.float16),
    }
    total = n_rows * n_cores
    xp = np.zeros((total, D), dtype=np.float32)
    xp[: len(x_rows)] = x_rows
    xh = xp.astype(np.float16)
    xl = (xp - xh.astype(np.float32)).astype(np.float16)
    in_maps = []
    for i in range(n_cores):
        m = dict(base)
        parts = []
        for arr in (xh, xl):
            ac = arr[i * n_rows:(i + 1) * n_rows]
            parts.append(np.ascontiguousarray(
                ac.reshape(nt, P, DT, P).transpose(0, 3, 2, 1)).reshape(nt, P, D))
        m["xt"] = np.concatenate(parts, axis=2)  # [nt, P, 2D]
        in_maps.append(m)
    return in_maps


def _run_exact_f16(x_rows: np.ndarray, centroids: np.ndarray, n_rows: int):
    nc = _cached_nc(("f16x3", n_rows), lambda: build_nc_f16x3(n_rows))
    in_maps = make_f16x3_in_maps(x_rows, centroids, n_rows)
    res = _run_spmd(nc, in_maps, "phase2")
    nt = n_rows // P
    parts = []
    for i in range(NCORES):
        ii = res.results[i]["outi"].reshape(P, nt, 8)
        parts.append(ii.transpose(1, 0, 2).reshape(n_rows, 8)[:, 0])
    out = np.concatenate(parts).astype(np.int32)
    return out[: len(x_rows)], res


def make_f16_in_maps(x: np.ndarray, centroids: np.ndarray,
                     n_rows: int = NSH, n_cores: int = NCORES):
    x = np.ascontiguousarray(x, dtype=np.float32)
    c = np.ascontiguousarray(centroids, dtype=np.float32)
    nt = n_rows // P
    c_norm = (c.astype(np.float64) ** 2).sum(axis=0)
    b = (384.0 - 0.5 * c_norm).astype(np.float32)  # pre-shifted bias
    b_hi = b.astype(np.float16)
    b_lo = (b - b_hi.astype(np.float32)).astype(np.float16)
    base = {
        "c": c.astype(np.float16),
        "bias2": np.stack([b_hi, b_lo]),
        "biasr": np.broadcast_to(b_hi, (P, K)).copy(),
        "ones": np.ones((2, P), dtype=np.float16),
    }
    x16 = x.astype(np.float16)
    in_maps = []
    for i in range(n_cores):
        xc = x16[i * n_rows:(i + 1) * n_rows]
        # xt[t, d, c, col] = x[t*128+col, c*128+d]
        xt = np.ascontiguousarray(
            xc.reshape(nt, P, DT, P).transpose(0, 3, 2, 1)).reshape(nt, P, D)
        m = dict(base)
        m["xt"] = xt
        in_maps.append(m)
    return in_maps


def build_nc(mode: str = "bf16x3", n_rows: int = NSH):
    """Build + compile the per-core Bass program.

    mode: 'bf16x3' (hi/lo split, 3 bf16 passes), 'fp32', 'fp32r', 'bf16'
    """
    assert n_rows % P == 0
    nt = n_rows // P
    nc = bacc.Bacc("TRN2", target_bir_lowering=False, debug=False)

    x = nc.dram_tensor("x", [n_rows, D], F32, kind="ExternalInput").ap()
    bias = nc.dram_tensor("bias", [P, K], F32, kind="ExternalInput").ap()
    out = nc.dram_tensor("out", [n_rows, 1], U32, kind="ExternalOutput").ap()

    split = mode == "bf16x3"
    cdt = BF if mode in ("bf16x3", "bf16") else F32
    mmdt = {"bf16x3": BF, "bf16": BF, "fp32": F32, "fp32r": mybir.dt.float32r}[mode]

    if split:
        c_hi = nc.dram_tensor("c_hi", [D, K], BF, kind="ExternalInput").ap()
        c_lo = nc.dram_tensor("c_lo", [D, K], BF, kind="ExternalInput").ap()
        c_srcs = [c_hi, c_lo]
    else:
        c_full = nc.dram_tensor("c", [D, K], cdt, kind="ExternalInput").ap()
        c_srcs = [c_full]
    ident = nc.dram_tensor("ident", [P, P], mmdt if mmdt != mybir.dt.float32r else F32,
                           kind="ExternalInput").ap()

    with tile.TileContext(nc) as tc, ExitStack() as ctx:
        const = ctx.enter_context(tc.tile_pool(name="const", bufs=1))
        xin_p = ctx.enter_context(tc.tile_pool(name="xin", bufs=3))
        xcast_p = ctx.enter_context(tc.tile_pool(name="xcast", bufs=2))
        xtp_p = ctx.enter_context(tc.tile_pool(name="xtp", bufs=2, space="PSUM"))
        xts_p = ctx.enter_context(tc.tile_pool(name="xts", bufs=2))
        sc_p = ctx.enter_context(tc.tile_pool(name="sc", bufs=1, space="PSUM"))
        ss_p = ctx.enter_context(tc.tile_pool(name="ss", bufs=2))
        mx_p = ctx.enter_context(tc.tile_pool(name="mx", bufs=4))

        # centroids resident in SBUF: [DT][P, K] per source (hi/lo or single)
        c_tiles = []
        for si, csrc in enumerate(c_srcs):
            c3 = csrc.rearrange("(t p) k -> t p k", p=P)
            tiles = []
            for d in range(DT):
                ct = const.tile([P, K], cdt, tag=f"c{si}_{d}")
                nc.sync.dma_start(ct[:], c3[d])
                tiles.append(ct)
            c_tiles.append(tiles)

        bias_t = const.tile([P, K], F32, tag="bias")
        nc.sync.dma_start(bias_t[:], bias[:, :])
        id_t = const.tile([P, P], ident.dtype, tag="ident")
        nc.sync.dma_start(id_t[:], ident[:, :])

        for t in range(nt):
            xin = xin_p.tile([P, D], F32)
            nc.scalar.dma_start(xin[:], x[t * P:(t + 1) * P, :])

            if split:
                xh = xcast_p.tile([P, D], BF, tag="xh")
                nc.scalar.copy(xh[:], xin[:])
                xl = xcast_p.tile([P, D], BF, tag="xl")
                nc.vector.tensor_sub(xl[:], xin[:], xh[:])
                tsrc = [xh, xl]
            elif mode == "bf16":
                xh = xcast_p.tile([P, D], BF, tag="xh")
                nc.scalar.copy(xh[:], xin[:])
                tsrc = [xh]
            else:
                tsrc = [xin]

            # transpose x tiles -> [d, n] layout for matmul weights
            nsrc = len(tsrc)
            tdt = BF if cdt == BF else F32
            xtp = xtp_p.tile([P, D * nsrc], tdt)
            for si, xsrc in enumerate(tsrc):
                for d in range(DT):
                    nc.tensor.transpose(
                        xtp[:, si * D + d * P: si * D + (d + 1) * P],
                        xsrc[:, d * P:(d + 1) * P],
                        id_t[:],
                    )
            xts = xts_p.tile([P, D * nsrc], tdt)
            nc.scalar.copy(xts[:], xtp[:])

            def w(si, d):
                return xts[:, si * D + d * P: si * D + (d + 1) * P]

            banks = []
            for b in range(NB):
                bank_tile = sc_p.tile([P, KW[b]], F32, tag=f"b{b}", name=f"bank{b}")
                banks.append(bank_tile)
            if split:
                # accumulate xh.ch + xh.cl + xl.ch over d
                for d in range(DT):
                    for b in range(NB):
                        nc.tensor.matmul(
                            banks[b][:], w(0, d),
                            c_tiles[0][d][:, KOFF[b]:KOFF[b] + KW[b]],
                            start=(d == 0), stop=False)
                    for b in range(NB):
                        nc.tensor.matmul(
                            banks[b][:], w(0, d),
                            c_tiles[1][d][:, KOFF[b]:KOFF[b] + KW[b]],
                            start=False, stop=False)
                    for b in range(NB):
                        nc.tensor.matmul(
                            banks[b][:], w(1, d),
                            c_tiles[0][d][:, KOFF[b]:KOFF[b] + KW[b]],
                            start=False, stop=(d == DT - 1))
            else:
                for d in range(DT):
                    for b in range(NB):
                        lhs = w(0, d)
                        rhs = c_tiles[0][d][:, KOFF[b]:KOFF[b] + KW[b]]
                        if mode == "fp32r":
                            lhs = lhs.bitcast(mybir.dt.float32r)
                            rhs = rhs.bitcast(mybir.dt.float32r)
                        nc.tensor.matmul(banks[b][:], lhs, rhs,
                                         start=(d == 0), stop=(d == DT - 1))

            ss = ss_p.tile([P, K], F32)
            for b in range(NB):
                nc.vector.tensor_add(
                    ss[:, KOFF[b]:KOFF[b] + KW[b]], banks[b][:],
                    bias_t[:, KOFF[b]:KOFF[b] + KW[b]])

            mxv = mx_p.tile([P, 8], F32, tag="mxv")
            nc.vector.max(mxv[:], ss[:])
            mxi = mx_p.tile([P, 8], U32, tag="mxi")
            nc.vector.max_index(mxi[:], mxv[:], ss[:])
            nc.scalar.dma_start(out[t * P:(t + 1) * P, :], mxi[:, 0:1])

    nc.compile()
    return nc


def make_in_maps(x: np.ndarray, centroids: np.ndarray, mode: str = "bf16x3",
                 n_rows: int = NSH, n_cores: int = NCORES):
    x = np.ascontiguousarray(x, dtype=np.float32)
    c = np.ascontiguousarray(centroids, dtype=np.float32)
    c_norm = (c.astype(np.float64) ** 2).sum(axis=0)
    bias = np.broadcast_to((-0.5 * c_norm).astype(np.float32), (P, K)).copy()

    base = {"bias": bias}
    if mode == "bf16x3":
        c_hi = c.astype(BF16)
        c_lo = (c - c_hi.astype(np.float32)).astype(BF16)
        base["c_hi"] = c_hi
        base["c_lo"] = c_lo
        base["ident"] = np.eye(P, dtype=BF16)
    elif mode == "bf16":
        base["c"] = c.astype(BF16)
        base["ident"] = np.eye(P, dtype=BF16)
    else:
        base["c"] = c
        base["ident"] = np.eye(P, dtype=np.float32)

    in_maps = []
    for i in range(n_cores):
        m = dict(base)
        m["x"] = x[i * n_rows:(i + 1) * n_rows]
        in_maps.append(m)
    return in_maps


_NC_CACHE = {}
LAST_RESULTS = []  # (label, BassKernelResults) of the most recent kernel() call


def _run_spmd(nc, in_maps, label):
    kw = {}
    if os.environ.get("KMEANS_TRACE"):
        import shutil

        kw["trace"] = True
        kw["tmpdir"] = os.environ.get("KMEANS_TRACE_DIR", "/tmp/km_trace") + "_" + label
        shutil.rmtree(kw["tmpdir"], ignore_errors=True)
        os.makedirs(kw["tmpdir"], exist_ok=True)
    res = run_bass_kernel_spmd(nc, in_maps, core_ids=list(range(NCORES)), **kw)
    LAST_RESULTS.append((label, res))
    return res

# Phase-2 capacity: rows per core recomputed exactly. Margin threshold:
# empirical max bf16 score error on randn data is ~0.2; flag anything under
# 4x that. ~5% of rows get flagged at this threshold.
P2_ROWS = 1024
MARGIN_TH = None  # set below after calibration constant


def _cached_nc(key, builder):
    if key not in _NC_CACHE:
        _NC_CACHE[key] = builder()
    return _NC_CACHE[key]


def make_screen_in_maps(x: np.ndarray, centroids: np.ndarray,
                        n_rows: int = NSH, n_cores: int = NCORES):
    x = np.ascontiguousarray(x, dtype=np.float32)
    c = np.ascontiguousarray(centroids, dtype=np.float32)
    c_norm = (c.astype(np.float64) ** 2).sum(axis=0)
    bias = (-0.5 * c_norm).astype(np.float32)
    bias_hi = bias.astype(BF16)
    bias_lo = (bias - bias_hi.astype(np.float32)).astype(BF16)
    base = {
        "c": c.astype(BF16),
        "bias2": np.stack([bias_hi, bias_lo]),
        "ones": np.ones((2, P), dtype=BF16),
        "ident": np.eye(P, dtype=BF16),
    }
    in_maps = []
    for i in range(n_cores):
        m = dict(base)
        m["x"] = x[i * n_rows:(i + 1) * n_rows]
        in_maps.append(m)
    return in_maps


def _run_exact(x_rows: np.ndarray, centroids: np.ndarray, n_rows: int):
    """Run the exact (bf16x3) program on x_rows padded to n_rows*NCORES."""
    nc = _cached_nc(("bf16x3", n_rows), lambda: build_nc("bf16x3", n_rows))
    total = n_rows * NCORES
    xp = np.zeros((total, D), dtype=np.float32)
    xp[: len(x_rows)] = x_rows
    in_maps = make_in_maps(xp, centroids, mode="bf16x3", n_rows=n_rows)
    res = _run_spmd(nc, in_maps, "phase2")
    out = np.concatenate(
        [res.results[i]["out"].reshape(n_rows) for i in range(NCORES)])
    return out[: len(x_rows)], res


def kernel(x: np.ndarray, centroids: np.ndarray) -> np.ndarray:
    mode = os.environ.get("KMEANS_MODE", "f16")
    LAST_RESULTS.clear()
    x = np.asarray(x)
    centroids = np.asarray(centroids)

    if mode == "f16":
        # phase 1: fp16 screen with top-8 values + indices
        nc1 = _cached_nc(("f16", NSH), lambda: build_nc_f16(NSH))
        in_maps = make_f16_in_maps(x, centroids)
        res1 = _run_spmd(nc1, in_maps, "phase1")
        nt = NSH // P
        idx_parts, val_parts = [], []
        for i in range(NCORES):
            vi = res1.results[i]["outv"].reshape(P, nt, 8).astype(np.float32)
            ii = res1.results[i]["outi"].reshape(P, nt, 8)
            # row n = t*128 + p  ->  [t, p]
            val_parts.append(vi.transpose(1, 0, 2).reshape(NSH, 8))
            idx_parts.append(ii.transpose(1, 0, 2).reshape(NSH, 8))
        vals = np.concatenate(val_parts)
        idx = np.concatenate(idx_parts)[:, 0].astype(np.int32)

        # fp16 values: quantization ~0.06-0.125 near |s'|<=256 plus fp16
        # matmul error ~0.05 -> th=0.3 flags everything at risk
        margin = vals[:, 0] - vals[:, 1]
        th = float(os.environ.get("KMEANS_MARGIN_TH", "0.3"))
        flagged = np.flatnonzero(margin < th)

        if os.environ.get("KMEANS_DEBUG"):
            print(f"[f16] flagged={len(flagged)} ({100 * len(flagged) / N:.2f}%)",
                  flush=True)
        sizes = [128, 256, 512, 1024, 2048]
        per_core = min((s for s in sizes if s * NCORES >= len(flagged)),
                       default=sizes[-1])
        cap = per_core * NCORES
        for s in range(0, len(flagged), cap):
            rows = flagged[s:s + cap]
            exact_idx, _ = _run_exact_f16(x[rows], centroids, per_core)
            idx[rows] = exact_idx
        return idx

    if mode != "hybrid":
        nc = _cached_nc((mode, NSH), lambda: build_nc(mode=mode))
        in_maps = make_in_maps(x, centroids, mode=mode)
        res = _run_spmd(nc, in_maps, mode)
        parts = [res.results[i]["out"].reshape(NSH) for i in range(NCORES)]
        return np.concatenate(parts).astype(np.int32)

    # phase 1: bf16 screen with top-2 margins
    nc1 = _cached_nc(("screen", NSH), lambda: build_nc_screen(NSH))
    in_maps = make_screen_in_maps(x, centroids)
    res1 = _run_spmd(nc1, in_maps, "phase1")
    idx = np.concatenate(
        [res1.results[i]["out"].reshape(NSH) for i in range(NCORES)]
    ).astype(np.int32)
    vals = np.concatenate(
        [res1.results[i]["vals"].reshape(NSH, 2) for i in range(NCORES)])

    margin = vals[:, 0] - vals[:, 1]
    th = float(os.environ.get("KMEANS_MARGIN_TH", "0.8"))
    flagged = np.flatnonzero(margin < th)

    # phase 2: exact recompute of flagged rows; pick the smallest padded
    # program that covers the count, chunking in the (unexpected) overflow case
    sizes = [512, 1024, 1536, 2048]
    per_core = min((s for s in sizes if s * NCORES >= len(flagged)),
                   default=sizes[-1])
    cap = per_core * NCORES
    for s in range(0, len(flagged), cap):
        rows = flagged[s:s + cap]
        exact_idx, _ = _run_exact(x[rows], centroids, per_core)
        idx[rows] = exact_idx
    return idx



# revision 12
# speedup vs baseline: 1.6600x; 1.0088x over previous
"""Trainium2 Bass kernel for KMeans assignment (argmin over centroid distances).

Problem: x [131072, 768] f32, centroids [768, 2000] f32
Output:  argmin_k ||x_n - c_k||^2  -> int32 [131072]

Math: argmin_k(||x||^2 - 2 x.c_k + ||c_k||^2) = argmax_k(x.c_k - 0.5||c_k||^2).
Per-core (data-parallel over 8 cores, 16384 rows each):
  - keep centroids resident in SBUF (bf16 hi/lo split for near-fp32 matmul
    precision at bf16 PE rate: x.c = xh.ch + xh.cl + xl.ch, dropping xl.cl)
  - per 128-row tile: DMA x, cast hi/lo, PE-transpose to [d, n] weights,
    matmul-accumulate scores into PSUM, DVE adds bias (-0.5||c||^2, broadcast
    from host), DVE max/max_index gives argmax along free axis.
"""

import os
import sys

for _p in ("/opt/trn_rl_repo",):
    if _p not in sys.path and os.path.isdir(_p):
        sys.path.insert(0, _p)

from contextlib import ExitStack

import numpy as np

import concourse.bass as bass
import concourse.tile as tile
from concourse import bacc, mybir
from concourse.bass_utils import run_bass_kernel_spmd

try:
    import ml_dtypes

    BF16 = np.dtype(ml_dtypes.bfloat16)
except ImportError:  # pragma: no cover
    BF16 = None

N, D, K = 131072, 768, 2000
NCORES = 8
NSH = N // NCORES  # 16384 rows per core
P = 128
DT = D // P  # 6 contraction tiles
# score chunks, each within one PSUM bank (<=512 fp32)
KOFF = [0, 512, 1024, 1536]
KW = [512, 512, 512, 464]
NB = 4

F32 = mybir.dt.float32
BF = mybir.dt.bfloat16
U32 = mybir.dt.uint32


def build_nc_screen(n_rows: int = NSH):
    """Phase-1 screening program: single bf16 matmul pass.

    Bias (-0.5||c||^2) is folded into the matmul as two extra contraction
    rows (ones-weights x [bias_hi; bias_lo]) so the vector engine only runs
    max/max_index. Outputs the argmax index and the top-2 score values per
    row; rows with a small top-2 margin get recomputed exactly in phase 2.
    """
    assert n_rows % P == 0
    nt = n_rows // P
    nc = bacc.Bacc("TRN2", target_bir_lowering=False, debug=False)

    x = nc.dram_tensor("x", [n_rows, D], F32, kind="ExternalInput").ap()
    c_in = nc.dram_tensor("c", [D, K], BF, kind="ExternalInput").ap()
    bias2 = nc.dram_tensor("bias2", [2, K], BF, kind="ExternalInput").ap()
    ones = nc.dram_tensor("ones", [2, P], BF, kind="ExternalInput").ap()
    ident = nc.dram_tensor("ident", [P, P], BF, kind="ExternalInput").ap()
    out = nc.dram_tensor("out", [n_rows, 1], U32, kind="ExternalOutput").ap()
    vals = nc.dram_tensor("vals", [n_rows, 2], F32, kind="ExternalOutput").ap()

    with tile.TileContext(nc) as tc, ExitStack() as ctx:
        const = ctx.enter_context(tc.tile_pool(name="const", bufs=1))
        xin_p = ctx.enter_context(tc.tile_pool(name="xin", bufs=3))
        xcast_p = ctx.enter_context(tc.tile_pool(name="xcast", bufs=2))
        xtp_p = ctx.enter_context(tc.tile_pool(name="xtp", bufs=1, space="PSUM"))
        xts_p = ctx.enter_context(tc.tile_pool(name="xts", bufs=2))
        sc_p = ctx.enter_context(tc.tile_pool(name="sc", bufs=1, space="PSUM"))
        ss_p = ctx.enter_context(tc.tile_pool(name="ss", bufs=2))
        mx_p = ctx.enter_context(tc.tile_pool(name="mx", bufs=4))

        c3 = c_in.rearrange("(t p) k -> t p k", p=P)
        c_tiles = []
        for d in range(DT):
            ct = const.tile([P, K], BF, tag=f"c_{d}")
            nc.sync.dma_start(ct[:], c3[d])
            c_tiles.append(ct)
        bias_t = const.tile([2, K], BF, tag="bias2")
        nc.sync.dma_start(bias_t[:], bias2[:, :])
        ones_t = const.tile([2, P], BF, tag="ones")
        nc.sync.dma_start(ones_t[:], ones[:, :])
        id_t = const.tile([P, P], BF, tag="ident")
        nc.sync.dma_start(id_t[:], ident[:, :])

        for t in range(nt):
            xin = xin_p.tile([P, D], F32)
            nc.scalar.dma_start(xin[:], x[t * P:(t + 1) * P, :])
            xh = xcast_p.tile([P, D], BF, tag="xh")
            nc.scalar.copy(xh[:], xin[:])
            xtp = xtp_p.tile([P, D], BF)
            for d in range(DT):
                nc.tensor.transpose(
                    xtp[:, d * P:(d + 1) * P], xh[:, d * P:(d + 1) * P], id_t[:])
            xts = xts_p.tile([P, D], BF)
            nc.scalar.copy(xts[:], xtp[:])

            banks = []
            for b in range(NB):
                # first three banks double-buffered: they are what the next
                # tile's first matmuls wait on (PSUM budget: 2+2+2+1 + 1 xtp)
                bank_tile = sc_p.tile([P, KW[b]], F32, tag=f"b{b}",
                                      name=f"bank{b}", bufs=(2 if b < 3 else 1))
                banks.append(bank_tile)
            # bias rows first so each bank's accumulation closes on d == DT-1
            for b in range(NB):
                nc.tensor.matmul(
                    banks[b][:], ones_t[:],
                    bias_t[:, KOFF[b]:KOFF[b] + KW[b]],
                    start=True, stop=False)
            for d in range(DT):
                for b in range(NB):
                    nc.tensor.matmul(
                        banks[b][:], xts[:, d * P:(d + 1) * P],
                        c_tiles[d][:, KOFF[b]:KOFF[b] + KW[b]],
                        start=False, stop=(d == DT - 1))

            ss = ss_p.tile([P, K], F32)
            for b in range(NB):
                nc.scalar.copy(ss[:, KOFF[b]:KOFF[b] + KW[b]], banks[b][:])

            mxv = mx_p.tile([P, 8], F32, tag="mxv")
            nc.vector.max(mxv[:], ss[:])
            mxi = mx_p.tile([P, 8], U32, tag="mxi")
            nc.vector.max_index(mxi[:], mxv[:], ss[:])
            nc.scalar.dma_start(out[t * P:(t + 1) * P, :], mxi[:, 0:1])
            nc.scalar.dma_start(vals[t * P:(t + 1) * P, :], mxv[:, 0:2])

    nc.compile()
    return nc


def build_nc_f16(n_rows: int = NSH):
    """fp16 screening program, v2 of the screen.

    Differences from build_nc_screen (bf16):
      - x arrives from the host pre-transposed AND pre-cast to fp16 in tile
        layout xt[t, d, c, col] = x[t*128+col, c*128+d]: no on-device cast,
        no PE transpose, no PSUM->SBUF staging copy. fp16 mm runs at bf16
        speed with ~8x less quantization error -> far fewer phase-2 rows.
      - scores accumulate into ONE 4-bank PSUM tile [128, 2000] so DVE
        max/max_index read the banks directly (no scalar ss copy).
      - outputs (top-8 values f32 + indices u16) collect in SBUF and ship
        as two DMAs at the end: nothing downstream blocks on DVE results,
        so the PE never idles (avoids HAM re-throttle to 1.2 GHz).
      - x DMA issued from the otherwise-idle gpsimd sequencer.
    """
    assert n_rows % P == 0
    nt = n_rows // P
    nc = bacc.Bacc("TRN2", target_bir_lowering=False, debug=False)

    F16 = mybir.dt.float16
    U16 = mybir.dt.uint16

    # bias2/biasr hold hi/lo resp. fp16 of (b + 384): the +384 recenters
    # scores near 0 so every fp16 intermediate keeps small ulp
    xt = nc.dram_tensor("xt", [nt, P, D], F16, kind="ExternalInput").ap()
    c_in = nc.dram_tensor("c", [D, K], F16, kind="ExternalInput").ap()
    bias2 = nc.dram_tensor("bias2", [2, K], F16, kind="ExternalInput").ap()
    biasr = nc.dram_tensor("biasr", [P, K], F16, kind="ExternalInput").ap()
    ones = nc.dram_tensor("ones", [2, P], F16, kind="ExternalInput").ap()
    outv = nc.dram_tensor("outv", [P, nt * 8], F16, kind="ExternalOutput").ap()
    outi = nc.dram_tensor("outi", [P, nt * 8], U16, kind="ExternalOutput").ap()

    with tile.TileContext(nc) as tc, ExitStack() as ctx:
        const = ctx.enter_context(tc.tile_pool(name="const", bufs=1))
        xin_p = ctx.enter_context(tc.tile_pool(name="xin", bufs=4))
        sc_p = ctx.enter_context(tc.tile_pool(name="sc", bufs=2, space="PSUM"))
        sf_p = ctx.enter_context(tc.tile_pool(name="sf", bufs=2))
        ob_p = ctx.enter_context(tc.tile_pool(name="ob", bufs=1))

        # small consts first so tile 0's bias matmul can start immediately
        bias_t = const.tile([2, K], F16, tag="bias2", name="bias2")
        nc.sync.dma_start(bias_t[:], bias2[:, :])
        ones_t = const.tile([2, P], F16, tag="ones", name="ones")
        nc.sync.dma_start(ones_t[:], ones[:, :])
        biasr_t = const.tile([P, K], F16, tag="biasr", name="biasr")
        nc.sync.dma_start(biasr_t[:], biasr[:, :])
        c3 = c_in.rearrange("(t p) k -> t p k", p=P)
        c_tiles = []
        for d in range(DT):
            ct = const.tile([P, K], F16, tag=f"c_{d}", name=f"c_{d}")
            nc.sync.dma_start(ct[:], c3[d])
            c_tiles.append(ct)

        outv_t = ob_p.tile([P, nt * 8], F16, tag="outv", name="outv_t")
        outi_t = ob_p.tile([P, nt * 8], U16, tag="outi", name="outi_t")

        for t in range(nt):
            xts = xin_p.tile([P, D], F16, name="xts")
            nc.gpsimd.dma_start(xts[:], xt[t])

            ss = sc_p.tile([P, K], F32, name="ss")
            # bias via PE ones-matmul for banks 0-1; banks 2-3 get the
            # (b+384) residual added by the DVE on the fp16 copy (2x rate).
            # Balances PE (~5.6us) vs DVE (~5.2us) and keeps the extra fp16
            # rounding (-> wider flagged margins) limited to half the k range.
            for b in range(2):
                nc.tensor.matmul(
                    ss[:, KOFF[b]:KOFF[b] + KW[b]], ones_t[:],
                    bias_t[:, KOFF[b]:KOFF[b] + KW[b]],
                    start=True, stop=False, skip_group_check=True)
            for d in range(DT):
                for b in range(NB):
                    nc.tensor.matmul(
                        ss[:, KOFF[b]:KOFF[b] + KW[b]],
                        xts[:, d * P:(d + 1) * P],
                        c_tiles[d][:, KOFF[b]:KOFF[b] + KW[b]],
                        start=(d == 0 and b > 1), stop=(d == DT - 1),
                        skip_group_check=True)

            # fp16 copy: fp16 max/max_index pairs run ~2x faster than
            # f32/PSUM ones (no inter-op pipe-drain bubbles)
            sf = sf_p.tile([P, K], F16, name="sf")
            nc.scalar.copy(sf[:], ss[:])
            nc.vector.tensor_add(sf[:, KOFF[2]:K], sf[:, KOFF[2]:K],
                                 biasr_t[:, KOFF[2]:K])

            vslot = outv_t[:, t * 8:(t + 1) * 8]
            nc.vector.max(vslot, sf[:])
            nc.vector.max_index(outi_t[:, t * 8:(t + 1) * 8], vslot, sf[:])

        nc.sync.dma_start(outv[:, :], outv_t[:])
        nc.sync.dma_start(outi[:, :], outi_t[:])

    nc.compile()
    return nc


def build_nc_f16x3(n_rows: int):
    """Phase-2 exact rescore, fp16 hi/lo 3-term (error ~3e-6 on randn data).

    Same pre-transposed tile layout as the f16 screen, but x ships as
    [xh; xl] (fp16 value + fp16 residual) and c as ch + cl constants.
    s = xh.ch + xh.cl + xl.ch (+bias), argmax via f32-PSUM max/max_index
    (fp16 reduction would blur the tight margins phase-2 rows have).
    """
    assert n_rows % P == 0
    nt = n_rows // P
    nc = bacc.Bacc("TRN2", target_bir_lowering=False, debug=False)

    F16 = mybir.dt.float16
    U16 = mybir.dt.uint16

    xt = nc.dram_tensor("xt", [nt, P, 2 * D], F16, kind="ExternalInput").ap()
    ch_in = nc.dram_tensor("ch", [D, K], F16, kind="ExternalInput").ap()
    cl_in = nc.dram_tensor("cl", [D, K], F16, kind="ExternalInput").ap()
    bias2 = nc.dram_tensor("bias2", [2, K], F16, kind="ExternalInput").ap()
    ones = nc.dram_tensor("ones", [2, P], F16, kind="ExternalInput").ap()
    outi = nc.dram_tensor("outi", [P, nt * 8], U16, kind="ExternalOutput").ap()

    with tile.TileContext(nc) as tc, ExitStack() as ctx:
        const = ctx.enter_context(tc.tile_pool(name="const", bufs=1))
        xin_p = ctx.enter_context(tc.tile_pool(name="xin", bufs=3))
        sc_p = ctx.enter_context(tc.tile_pool(name="sc", bufs=2, space="PSUM"))
        mx_p = ctx.enter_context(tc.tile_pool(name="mx", bufs=2))
        ob_p = ctx.enter_context(tc.tile_pool(name="ob", bufs=1))

        bias_t = const.tile([2, K], F16, tag="bias2", name="bias2")
        nc.sync.dma_start(bias_t[:], bias2[:, :])
        ones_t = const.tile([2, P], F16, tag="ones", name="ones")
        nc.sync.dma_start(ones_t[:], ones[:, :])
        c_tiles = {}
        for nm, src in (("ch", ch_in), ("cl", cl_in)):
            c3 = src.rearrange("(t p) k -> t p k", p=P)
            for d in range(DT):
                ct = const.tile([P, K], F16, tag=f"{nm}_{d}", name=f"{nm}_{d}")
                nc.sync.dma_start(ct[:], c3[d])
                c_tiles[(nm, d)] = ct

        outi_t = ob_p.tile([P, nt * 8], U16, tag="outi", name="outi_t")

        for t in range(nt):
            xts = xin_p.tile([P, 2 * D], F16, name="xts")
            nc.gpsimd.dma_start(xts[:], xt[t])

            def w(h, d):  # stationary x chunk: h=0 hi, h=1 lo
                off = h * D + d * P
                return xts[:, off:off + P]

            ss = sc_p.tile([P, K], F32, name="ss")
            for b in range(NB):
                nc.tensor.matmul(
                    ss[:, KOFF[b]:KOFF[b] + KW[b]], ones_t[:],
                    bias_t[:, KOFF[b]:KOFF[b] + KW[b]],
                    start=True, stop=False, skip_group_check=True)
            terms = [(0, "ch"), (0, "cl"), (1, "ch")]
            for ti, (h, cn) in enumerate(terms):
                for d in range(DT):
                    last = ti == len(terms) - 1 and d == DT - 1
                    for b in range(NB):
                        nc.tensor.matmul(
                            ss[:, KOFF[b]:KOFF[b] + KW[b]], w(h, d),
                            c_tiles[(cn, d)][:, KOFF[b]:KOFF[b] + KW[b]],
                            start=False, stop=last, skip_group_check=True)

            mxv = mx_p.tile([P, 8], F32, name="mxv")
            nc.vector.max(mxv[:], ss[:])
            nc.vector.max_index(outi_t[:, t * 8:(t + 1) * 8], mxv[:], ss[:])

        nc.sync.dma_start(outi[:, :], outi_t[:])

    nc.compile()
    return nc


def make_f16x3_in_maps(x_rows: np.ndarray, centroids: np.ndarray,
                       n_rows: int, n_cores: int = NCORES):
    """x_rows: gathered flagged rows [m, D] f32, padded to n_rows*n_cores."""
    c = np.ascontiguousarray(centroids, dtype=np.float32)
    nt = n_rows // P
    ch = c.astype(np.float16)
    cl = (c - ch.astype(np.float32)).astype(np.float16)
    c_norm = (c.astype(np.float64) ** 2).sum(axis=0)
    b = (-0.5 * c_norm).astype(np.float32)
    b_hi = b.astype(np.float16)
    b_lo = (b - b_hi.astype(np.float32)).astype(np.float16)
    base = {
        "ch": ch, "cl": cl,
        "bias2": np.stack([b_hi, b_lo]),
        "ones": np.ones((2, P), dtype=np.float16),
    }
    total = n_rows * n_cores
    xp = np.zeros((total, D), dtype=np.float32)
    xp[: len(x_rows)] = x_rows
    xh = xp.astype(np.float16)
    xl = (xp - xh.astype(np.float32)).astype(np.float16)
    in_maps = []
    for i in range(n_cores):
        m = dict(base)
        parts = []
        for arr in (xh, xl):
            ac = arr[i * n_rows:(i + 1) * n_rows]
            parts.append(np.ascontiguousarray(
                ac.reshape(nt, P, DT, P).transpose(0, 3, 2, 1)).reshape(nt, P, D))
        m["xt"] = np.concatenate(parts, axis=2)  # [nt, P, 2D]
        in_maps.append(m)
    return in_maps


def _run_exact_f16(x_rows: np.ndarray, centroids: np.ndarray, n_rows: int):
    nc = _cached_nc(("f16x3", n_rows), lambda: build_nc_f16x3(n_rows))
    in_maps = make_f16x3_in_maps(x_rows, centroids, n_rows)
    res = _run_spmd(nc, in_maps, "phase2")
    nt = n_rows // P
    parts = []
    for i in range(NCORES):
        ii = res.results[i]["outi"].reshape(P, nt, 8)
        parts.append(ii.transpose(1, 0, 2).reshape(n_rows, 8)[:, 0])
    out = np.concatenate(parts).astype(np.int32)
    return out[: len(x_rows)], res


def make_f16_in_maps(x: np.ndarray, centroids: np.ndarray,
                     n_rows: int = NSH, n_cores: int = NCORES):
    x = np.ascontiguousarray(x, dtype=np.float32)
    c = np.ascontiguousarray(centroids, dtype=np.float32)
    nt = n_rows // P
    c_norm = (c.astype(np.float64) ** 2).sum(axis=0)
    b = (384.0 - 0.5 * c_norm).astype(np.float32)  # pre-shifted bias
    b_hi = b.astype(np.float16)
    b_lo = (b - b_hi.astype(np.float32)).astype(np.float16)
    base = {
        "c": c.astype(np.float16),
        "bias2": np.stack([b_hi, b_lo]),
        "biasr": np.broadcast_to(b_hi, (P, K)).copy(),
        "ones": np.ones((2, P), dtype=np.float16),
    }
    x16 = x.astype(np.float16)
    in_maps = []
    for i in range(n_cores):
        xc = x16[i * n_rows:(i + 1) * n_rows]
        # xt[t, d, c, col] = x[t*128+col, c*128+d]
        xt = np.ascontiguousarray(
            xc.reshape(nt, P, DT, P).transpose(0, 3, 2, 1)).reshape(nt, P, D)
        m = dict(base)
        m["xt"] = xt
        in_maps.append(m)
    return in_maps


def build_nc(mode: str = "bf16x3", n_rows: int = NSH):
    """Build + compile the per-core Bass program.

    mode: 'bf16x3' (hi/lo split, 3 bf16 passes), 'fp32', 'fp32r', 'bf16'
    """
    assert n_rows % P == 0
    nt = n_rows // P
    nc = bacc.Bacc("TRN2", target_bir_lowering=False, debug=False)

    x = nc.dram_tensor("x", [n_rows, D], F32, kind="ExternalInput").ap()
    bias = nc.dram_tensor("bias", [P, K], F32, kind="ExternalInput").ap()
    out = nc.dram_tensor("out", [n_rows, 1], U32, kind="ExternalOutput").ap()

    split = mode == "bf16x3"
    cdt = BF if mode in ("bf16x3", "bf16") else F32
    mmdt = {"bf16x3": BF, "bf16": BF, "fp32": F32, "fp32r": mybir.dt.float32r}[mode]

    if split:
        c_hi = nc.dram_tensor("c_hi", [D, K], BF, kind="ExternalInput").ap()
        c_lo = nc.dram_tensor("c_lo", [D, K], BF, kind="ExternalInput").ap()
        c_srcs = [c_hi, c_lo]
    else:
        c_full = nc.dram_tensor("c", [D, K], cdt, kind="ExternalInput").ap()
        c_srcs = [c_full]
    ident = nc.dram_tensor("ident", [P, P], mmdt if mmdt != mybir.dt.float32r else F32,
                           kind="ExternalInput").ap()

    with tile.TileContext(nc) as tc, ExitStack() as ctx:
        const = ctx.enter_context(tc.tile_pool(name="const", bufs=1))
        xin_p = ctx.enter_context(tc.tile_pool(name="xin", bufs=3))
        xcast_p = ctx.enter_context(tc.tile_pool(name="xcast", bufs=2))
        xtp_p = ctx.enter_context(tc.tile_pool(name="xtp", bufs=2, space="PSUM"))
        xts_p = ctx.enter_context(tc.tile_pool(name="xts", bufs=2))
        sc_p = ctx.enter_context(tc.tile_pool(name="sc", bufs=1, space="PSUM"))
        ss_p = ctx.enter_context(tc.tile_pool(name="ss", bufs=2))
        mx_p = ctx.enter_context(tc.tile_pool(name="mx", bufs=4))

        # centroids resident in SBUF: [DT][P, K] per source (hi/lo or single)
        c_tiles = []
        for si, csrc in enumerate(c_srcs):
            c3 = csrc.rearrange("(t p) k -> t p k", p=P)
            tiles = []
            for d in range(DT):
                ct = const.tile([P, K], cdt, tag=f"c{si}_{d}")
                nc.sync.dma_start(ct[:], c3[d])
                tiles.append(ct)
            c_tiles.append(tiles)

        bias_t = const.tile([P, K], F32, tag="bias")
        nc.sync.dma_start(bias_t[:], bias[:, :])
        id_t = const.tile([P, P], ident.dtype, tag="ident")
        nc.sync.dma_start(id_t[:], ident[:, :])

        for t in range(nt):
            xin = xin_p.tile([P, D], F32)
            nc.scalar.dma_start(xin[:], x[t * P:(t + 1) * P, :])

            if split:
                xh = xcast_p.tile([P, D], BF, tag="xh")
                nc.scalar.copy(xh[:], xin[:])
                xl = xcast_p.tile([P, D], BF, tag="xl")
                nc.vector.tensor_sub(xl[:], xin[:], xh[:])
                tsrc = [xh, xl]
            elif mode == "bf16":
                xh = xcast_p.tile([P, D], BF, tag="xh")
                nc.scalar.copy(xh[:], xin[:])
                tsrc = [xh]
            else:
                tsrc = [xin]

            # transpose x tiles -> [d, n] layout for matmul weights
            nsrc = len(tsrc)
            tdt = BF if cdt == BF else F32
            xtp = xtp_p.tile([P, D * nsrc], tdt)
            for si, xsrc in enumerate(tsrc):
                for d in range(DT):
                    nc.tensor.transpose(
                        xtp[:, si * D + d * P: si * D + (d + 1) * P],
                        xsrc[:, d * P:(d + 1) * P],
                        id_t[:],
                    )
            xts = xts_p.tile([P, D * nsrc], tdt)
            nc.scalar.copy(xts[:], xtp[:])

            def w(si, d):
                return xts[:, si * D + d * P: si * D + (d + 1) * P]

            banks = []
            for b in range(NB):
                bank_tile = sc_p.tile([P, KW[b]], F32, tag=f"b{b}", name=f"bank{b}")
                banks.append(bank_tile)
            if split:
                # accumulate xh.ch + xh.cl + xl.ch over d
                for d in range(DT):
                    for b in range(NB):
                        nc.tensor.matmul(
                            banks[b][:], w(0, d),
                            c_tiles[0][d][:, KOFF[b]:KOFF[b] + KW[b]],
                            start=(d == 0), stop=False)
                    for b in range(NB):
                        nc.tensor.matmul(
                            banks[b][:], w(0, d),
                            c_tiles[1][d][:, KOFF[b]:KOFF[b] + KW[b]],
                            start=False, stop=False)
                    for b in range(NB):
                        nc.tensor.matmul(
                            banks[b][:], w(1, d),
                            c_tiles[0][d][:, KOFF[b]:KOFF[b] + KW[b]],
                            start=False, stop=(d == DT - 1))
            else:
                for d in range(DT):
                    for b in range(NB):
                        lhs = w(0, d)
                        rhs = c_tiles[0][d][:, KOFF[b]:KOFF[b] + KW[b]]
                        if mode == "fp32r":
                            lhs = lhs.bitcast(mybir.dt.float32r)
                            rhs = rhs.bitcast(mybir.dt.float32r)
                        nc.tensor.matmul(banks[b][:], lhs, rhs,
                                         start=(d == 0), stop=(d == DT - 1))

            ss = ss_p.tile([P, K], F32)
            for b in range(NB):
                nc.vector.tensor_add(
                    ss[:, KOFF[b]:KOFF[b] + KW[b]], banks[b][:],
                    bias_t[:, KOFF[b]:KOFF[b] + KW[b]])

            mxv = mx_p.tile([P, 8], F32, tag="mxv")
            nc.vector.max(mxv[:], ss[:])
            mxi = mx_p.tile([P, 8], U32, tag="mxi")
            nc.vector.max_index(mxi[:], mxv[:], ss[:])
            nc.scalar.dma_start(out[t * P:(t + 1) * P, :], mxi[:, 0:1])

    nc.compile()
    return nc


def make_in_maps(x: np.ndarray, centroids: np.ndarray, mode: str = "bf16x3",
                 n_rows: int = NSH, n_cores: int = NCORES):
    x = np.ascontiguousarray(x, dtype=np.float32)
    c = np.ascontiguousarray(centroids, dtype=np.float32)
    c_norm = (c.astype(np.float64) ** 2).sum(axis=0)
    bias = np.broadcast_to((-0.5 * c_norm).astype(np.float32), (P, K)).copy()

    base = {"bias": bias}
    if mode == "bf16x3":
        c_hi = c.astype(BF16)
        c_lo = (c - c_hi.astype(np.float32)).astype(BF16)
        base["c_hi"] = c_hi
        base["c_lo"] = c_lo
        base["ident"] = np.eye(P, dtype=BF16)
    elif mode == "bf16":
        base["c"] = c.astype(BF16)
        base["ident"] = np.eye(P, dtype=BF16)
    else:
        base["c"] = c
        base["ident"] = np.eye(P, dtype=np.float32)

    in_maps = []
    for i in range(n_cores):
        m = dict(base)
        m["x"] = x[i * n_rows:(i + 1) * n_rows]
        in_maps.append(m)
    return in_maps


_NC_CACHE = {}
LAST_RESULTS = []  # (label, BassKernelResults) of the most recent kernel() call


def _run_spmd(nc, in_maps, label):
    kw = {}
    if os.environ.get("KMEANS_TRACE"):
        import shutil

        kw["trace"] = True
        kw["tmpdir"] = os.environ.get("KMEANS_TRACE_DIR", "/tmp/km_trace") + "_" + label
        shutil.rmtree(kw["tmpdir"], ignore_errors=True)
        os.makedirs(kw["tmpdir"], exist_ok=True)
    res = run_bass_kernel_spmd(nc, in_maps, core_ids=list(range(NCORES)), **kw)
    LAST_RESULTS.append((label, res))
    return res

# Phase-2 capacity: rows per core recomputed exactly. Margin threshold:
# empirical max bf16 score error on randn data is ~0.2; flag anything under
# 4x that. ~5% of rows get flagged at this threshold.
P2_ROWS = 1024
MARGIN_TH = None  # set below after calibration constant


def _cached_nc(key, builder):
    if key not in _NC_CACHE:
        _NC_CACHE[key] = builder()
    return _NC_CACHE[key]


def make_screen_in_maps(x: np.ndarray, centroids: np.ndarray,
                        n_rows: int = NSH, n_cores: int = NCORES):
    x = np.ascontiguousarray(x, dtype=np.float32)
    c = np.ascontiguousarray(centroids, dtype=np.float32)
    c_norm = (c.astype(np.float64) ** 2).sum(axis=0)
    bias = (-0.5 * c_norm).astype(np.float32)
    bias_hi = bias.astype(BF16)
    bias_lo = (bias - bias_hi.astype(np.float32)).astype(BF16)
    base = {
        "c": c.astype(BF16),
        "bias2": np.stack([bias_hi, bias_lo]),
        "ones": np.ones((2, P), dtype=BF16),
        "ident": np.eye(P, dtype=BF16),
    }
    in_maps = []
    for i in range(n_cores):
        m = dict(base)
        m["x"] = x[i * n_rows:(i + 1) * n_rows]
        in_maps.append(m)
    return in_maps


def _run_exact(x_rows: np.ndarray, centroids: np.ndarray, n_rows: int):
    """Run the exact (bf16x3) program on x_rows padded to n_rows*NCORES."""
    nc = _cached_nc(("bf16x3", n_rows), lambda: build_nc("bf16x3", n_rows))
    total = n_rows * NCORES
    xp = np.zeros((total, D), dtype=np.float32)
    xp[: len(x_rows)] = x_rows
    in_maps = make_in_maps(xp, centroids, mode="bf16x3", n_rows=n_rows)
    res = _run_spmd(nc, in_maps, "phase2")
    out = np.concatenate(
        [res.results[i]["out"].reshape(n_rows) for i in range(NCORES)])
    return out[: len(x_rows)], res


def kernel(x: np.ndarray, centroids: np.ndarray) -> np.ndarray:
    mode = os.environ.get("KMEANS_MODE", "f16")
    LAST_RESULTS.clear()
    x = np.asarray(x)
    centroids = np.asarray(centroids)

    if mode == "f16":
        # phase 1: fp16 screen with top-8 values + indices
        nc1 = _cached_nc(("f16", NSH), lambda: build_nc_f16(NSH))
        in_maps = make_f16_in_maps(x, centroids)
        res1 = _run_spmd(nc1, in_maps, "phase1")
        nt = NSH // P
        idx_parts, val_parts = [], []
        for i in range(NCORES):
            vi = res1.results[i]["outv"].reshape(P, nt, 8).astype(np.float32)
            ii = res1.results[i]["outi"].reshape(P, nt, 8)
            # row n = t*128 + p  ->  [t, p]
            val_parts.append(vi.transpose(1, 0, 2).reshape(NSH, 8))
            idx_parts.append(ii.transpose(1, 0, 2).reshape(NSH, 8))
        vals = np.concatenate(val_parts)
        idx = np.concatenate(idx_parts)[:, 0].astype(np.int32)

        # fp16 values: quantization ~0.06-0.125 near |s'|<=256 plus fp16
        # matmul error ~0.05 -> th=0.3 flags everything at risk
        margin = vals[:, 0] - vals[:, 1]
        th = float(os.environ.get("KMEANS_MARGIN_TH", "0.3"))
        flagged = np.flatnonzero(margin < th)

        if os.environ.get("KMEANS_DEBUG"):
            print(f"[f16] flagged={len(flagged)} ({100 * len(flagged) / N:.2f}%)",
                  flush=True)
        sizes = [128, 256, 512, 768, 1024, 2048]
        per_core = min((s for s in sizes if s * NCORES >= len(flagged)),
                       default=sizes[-1])
        cap = per_core * NCORES
        for s in range(0, len(flagged), cap):
            rows = flagged[s:s + cap]
            exact_idx, _ = _run_exact_f16(x[rows], centroids, per_core)
            idx[rows] = exact_idx
        return idx

    if mode != "hybrid":
        nc = _cached_nc((mode, NSH), lambda: build_nc(mode=mode))
        in_maps = make_in_maps(x, centroids, mode=mode)
        res = _run_spmd(nc, in_maps, mode)
        parts = [res.results[i]["out"].reshape(NSH) for i in range(NCORES)]
        return np.concatenate(parts).astype(np.int32)

    # phase 1: bf16 screen with top-2 margins
    nc1 = _cached_nc(("screen", NSH), lambda: build_nc_screen(NSH))
    in_maps = make_screen_in_maps(x, centroids)
    res1 = _run_spmd(nc1, in_maps, "phase1")
    idx = np.concatenate(
        [res1.results[i]["out"].reshape(NSH) for i in range(NCORES)]
    ).astype(np.int32)
    vals = np.concatenate(
        [res1.results[i]["vals"].reshape(NSH, 2) for i in range(NCORES)])

    margin = vals[:, 0] - vals[:, 1]
    th = float(os.environ.get("KMEANS_MARGIN_TH", "0.8"))
    flagged = np.flatnonzero(margin < th)

    # phase 2: exact recompute of flagged rows; pick the smallest padded
    # program that covers the count, chunking in the (unexpected) overflow case
    sizes = [512, 1024, 1536, 2048]
    per_core = min((s for s in sizes if s * NCORES >= len(flagged)),
                   default=sizes[-1])
    cap = per_core * NCORES
    for s in range(0, len(flagged), cap):
        rows = flagged[s:s + cap]
        exact_idx, _ = _run_exact(x[rows], centroids, per_core)
        idx[rows] = exact_idx
    return idx



# revision 13
# speedup vs baseline: 1.7953x; 1.0815x over previous
"""Trainium2 Bass kernel for KMeans assignment (argmin over centroid distances).

Problem: x [131072, 768] f32, centroids [768, 2000] f32
Output:  argmin_k ||x_n - c_k||^2  -> int32 [131072]

Math: argmin_k(||x||^2 - 2 x.c_k + ||c_k||^2) = argmax_k(x.c_k - 0.5||c_k||^2).
Per-core (data-parallel over 8 cores, 16384 rows each):
  - keep centroids resident in SBUF (bf16 hi/lo split for near-fp32 matmul
    precision at bf16 PE rate: x.c = xh.ch + xh.cl + xl.ch, dropping xl.cl)
  - per 128-row tile: DMA x, cast hi/lo, PE-transpose to [d, n] weights,
    matmul-accumulate scores into PSUM, DVE adds bias (-0.5||c||^2, broadcast
    from host), DVE max/max_index gives argmax along free axis.
"""

import os
import sys

for _p in ("/opt/trn_rl_repo",):
    if _p not in sys.path and os.path.isdir(_p):
        sys.path.insert(0, _p)

from contextlib import ExitStack

import numpy as np

import concourse.bass as bass
import concourse.tile as tile
from concourse import bacc, mybir
from concourse.bass_utils import run_bass_kernel_spmd

try:
    import ml_dtypes

    BF16 = np.dtype(ml_dtypes.bfloat16)
except ImportError:  # pragma: no cover
    BF16 = None

N, D, K = 131072, 768, 2000
NCORES = 8
NSH = N // NCORES  # 16384 rows per core
P = 128
DT = D // P  # 6 contraction tiles
# score chunks, each within one PSUM bank (<=512 fp32)
KOFF = [0, 512, 1024, 1536]
KW = [512, 512, 512, 464]
NB = 4

F32 = mybir.dt.float32
BF = mybir.dt.bfloat16
U32 = mybir.dt.uint32


def build_nc_screen(n_rows: int = NSH):
    """Phase-1 screening program: single bf16 matmul pass.

    Bias (-0.5||c||^2) is folded into the matmul as two extra contraction
    rows (ones-weights x [bias_hi; bias_lo]) so the vector engine only runs
    max/max_index. Outputs the argmax index and the top-2 score values per
    row; rows with a small top-2 margin get recomputed exactly in phase 2.
    """
    assert n_rows % P == 0
    nt = n_rows // P
    nc = bacc.Bacc("TRN2", target_bir_lowering=False, debug=False)

    x = nc.dram_tensor("x", [n_rows, D], F32, kind="ExternalInput").ap()
    c_in = nc.dram_tensor("c", [D, K], BF, kind="ExternalInput").ap()
    bias2 = nc.dram_tensor("bias2", [2, K], BF, kind="ExternalInput").ap()
    ones = nc.dram_tensor("ones", [2, P], BF, kind="ExternalInput").ap()
    ident = nc.dram_tensor("ident", [P, P], BF, kind="ExternalInput").ap()
    out = nc.dram_tensor("out", [n_rows, 1], U32, kind="ExternalOutput").ap()
    vals = nc.dram_tensor("vals", [n_rows, 2], F32, kind="ExternalOutput").ap()

    with tile.TileContext(nc) as tc, ExitStack() as ctx:
        const = ctx.enter_context(tc.tile_pool(name="const", bufs=1))
        xin_p = ctx.enter_context(tc.tile_pool(name="xin", bufs=3))
        xcast_p = ctx.enter_context(tc.tile_pool(name="xcast", bufs=2))
        xtp_p = ctx.enter_context(tc.tile_pool(name="xtp", bufs=1, space="PSUM"))
        xts_p = ctx.enter_context(tc.tile_pool(name="xts", bufs=2))
        sc_p = ctx.enter_context(tc.tile_pool(name="sc", bufs=1, space="PSUM"))
        ss_p = ctx.enter_context(tc.tile_pool(name="ss", bufs=2))
        mx_p = ctx.enter_context(tc.tile_pool(name="mx", bufs=4))

        c3 = c_in.rearrange("(t p) k -> t p k", p=P)
        c_tiles = []
        for d in range(DT):
            ct = const.tile([P, K], BF, tag=f"c_{d}")
            nc.sync.dma_start(ct[:], c3[d])
            c_tiles.append(ct)
        bias_t = const.tile([2, K], BF, tag="bias2")
        nc.sync.dma_start(bias_t[:], bias2[:, :])
        ones_t = const.tile([2, P], BF, tag="ones")
        nc.sync.dma_start(ones_t[:], ones[:, :])
        id_t = const.tile([P, P], BF, tag="ident")
        nc.sync.dma_start(id_t[:], ident[:, :])

        for t in range(nt):
            xin = xin_p.tile([P, D], F32)
            nc.scalar.dma_start(xin[:], x[t * P:(t + 1) * P, :])
            xh = xcast_p.tile([P, D], BF, tag="xh")
            nc.scalar.copy(xh[:], xin[:])
            xtp = xtp_p.tile([P, D], BF)
            for d in range(DT):
                nc.tensor.transpose(
                    xtp[:, d * P:(d + 1) * P], xh[:, d * P:(d + 1) * P], id_t[:])
            xts = xts_p.tile([P, D], BF)
            nc.scalar.copy(xts[:], xtp[:])

            banks = []
            for b in range(NB):
                # first three banks double-buffered: they are what the next
                # tile's first matmuls wait on (PSUM budget: 2+2+2+1 + 1 xtp)
                bank_tile = sc_p.tile([P, KW[b]], F32, tag=f"b{b}",
                                      name=f"bank{b}", bufs=(2 if b < 3 else 1))
                banks.append(bank_tile)
            # bias rows first so each bank's accumulation closes on d == DT-1
            for b in range(NB):
                nc.tensor.matmul(
                    banks[b][:], ones_t[:],
                    bias_t[:, KOFF[b]:KOFF[b] + KW[b]],
                    start=True, stop=False)
            for d in range(DT):
                for b in range(NB):
                    nc.tensor.matmul(
                        banks[b][:], xts[:, d * P:(d + 1) * P],
                        c_tiles[d][:, KOFF[b]:KOFF[b] + KW[b]],
                        start=False, stop=(d == DT - 1))

            ss = ss_p.tile([P, K], F32)
            for b in range(NB):
                nc.scalar.copy(ss[:, KOFF[b]:KOFF[b] + KW[b]], banks[b][:])

            mxv = mx_p.tile([P, 8], F32, tag="mxv")
            nc.vector.max(mxv[:], ss[:])
            mxi = mx_p.tile([P, 8], U32, tag="mxi")
            nc.vector.max_index(mxi[:], mxv[:], ss[:])
            nc.scalar.dma_start(out[t * P:(t + 1) * P, :], mxi[:, 0:1])
            nc.scalar.dma_start(vals[t * P:(t + 1) * P, :], mxv[:, 0:2])

    nc.compile()
    return nc


def build_nc_f16(n_rows: int = NSH):
    """fp16 screening program, v2 of the screen.

    Differences from build_nc_screen (bf16):
      - x arrives from the host pre-transposed AND pre-cast to fp16 in tile
        layout xt[t, d, c, col] = x[t*128+col, c*128+d]: no on-device cast,
        no PE transpose, no PSUM->SBUF staging copy. fp16 mm runs at bf16
        speed with ~8x less quantization error -> far fewer phase-2 rows.
      - scores accumulate into ONE 4-bank PSUM tile [128, 2000] so DVE
        max/max_index read the banks directly (no scalar ss copy).
      - outputs (top-8 values f32 + indices u16) collect in SBUF and ship
        as two DMAs at the end: nothing downstream blocks on DVE results,
        so the PE never idles (avoids HAM re-throttle to 1.2 GHz).
      - x DMA issued from the otherwise-idle gpsimd sequencer.
    """
    assert n_rows % P == 0
    nt = n_rows // P
    nc = bacc.Bacc("TRN2", target_bir_lowering=False, debug=False)

    F16 = mybir.dt.float16
    U16 = mybir.dt.uint16

    # bias2/biasr hold hi/lo resp. fp16 of (b + 384): the +384 recenters
    # scores near 0 so every fp16 intermediate keeps small ulp
    xt = nc.dram_tensor("xt", [nt, P, D], F16, kind="ExternalInput").ap()
    c_in = nc.dram_tensor("c", [D, K], F16, kind="ExternalInput").ap()
    bias2 = nc.dram_tensor("bias2", [2, K], F16, kind="ExternalInput").ap()
    biasr = nc.dram_tensor("biasr", [P, K], F16, kind="ExternalInput").ap()
    ones = nc.dram_tensor("ones", [2, P], F16, kind="ExternalInput").ap()
    outv = nc.dram_tensor("outv", [P, nt * 8], F16, kind="ExternalOutput").ap()
    outi = nc.dram_tensor("outi", [P, nt * 8], U16, kind="ExternalOutput").ap()

    with tile.TileContext(nc) as tc, ExitStack() as ctx:
        const = ctx.enter_context(tc.tile_pool(name="const", bufs=1))
        xin_p = ctx.enter_context(tc.tile_pool(name="xin", bufs=4))
        sc_p = ctx.enter_context(tc.tile_pool(name="sc", bufs=2, space="PSUM"))
        sf_p = ctx.enter_context(tc.tile_pool(name="sf", bufs=2))
        ob_p = ctx.enter_context(tc.tile_pool(name="ob", bufs=1))

        # small consts first so tile 0's bias matmul can start immediately
        bias_t = const.tile([2, K], F16, tag="bias2", name="bias2")
        nc.sync.dma_start(bias_t[:], bias2[:, :])
        ones_t = const.tile([2, P], F16, tag="ones", name="ones")
        nc.sync.dma_start(ones_t[:], ones[:, :])
        c3 = c_in.rearrange("(t p) k -> t p k", p=P)
        c_tiles = []
        for d in range(DT):
            ct = const.tile([P, K], F16, tag=f"c_{d}", name=f"c_{d}")
            nc.sync.dma_start(ct[:], c3[d])
            c_tiles.append(ct)
        biasr_t = const.tile([P, K], F16, tag="biasr", name="biasr")
        nc.sync.dma_start(biasr_t[:], biasr[:, :])

        outv_t = ob_p.tile([P, nt * 8], F16, tag="outv", name="outv_t")
        outi_t = ob_p.tile([P, nt * 8], U16, tag="outi", name="outi_t")

        for t in range(nt):
            xts = xin_p.tile([P, D], F16, name="xts")
            nc.gpsimd.dma_start(xts[:], xt[t])

            ss = sc_p.tile([P, K], F32, name="ss")
            # bias via PE ones-matmul for bank 0 only; banks 1-3 get the
            # (b+384) residual added by the DVE on the fp16 copy (2x rate),
            # balancing PE (~5.43us) against DVE (~5.3us) per tile
            nc.tensor.matmul(
                ss[:, 0:KW[0]], ones_t[:], bias_t[:, 0:KW[0]],
                start=True, stop=False, skip_group_check=True)
            for d in range(DT):
                for b in range(NB):
                    nc.tensor.matmul(
                        ss[:, KOFF[b]:KOFF[b] + KW[b]],
                        xts[:, d * P:(d + 1) * P],
                        c_tiles[d][:, KOFF[b]:KOFF[b] + KW[b]],
                        start=(d == 0 and b > 0), stop=(d == DT - 1),
                        skip_group_check=True)

            # fp16 copy: fp16 max/max_index pairs run ~2x faster than
            # f32/PSUM ones (no inter-op pipe-drain bubbles)
            sf = sf_p.tile([P, K], F16, name="sf")
            nc.scalar.copy(sf[:], ss[:])
            nc.vector.tensor_add(sf[:, KW[0]:K], sf[:, KW[0]:K],
                                 biasr_t[:, KW[0]:K])

            vslot = outv_t[:, t * 8:(t + 1) * 8]
            nc.vector.max(vslot, sf[:])
            nc.vector.max_index(outi_t[:, t * 8:(t + 1) * 8], vslot, sf[:])

        nc.sync.dma_start(outv[:, :], outv_t[:])
        nc.sync.dma_start(outi[:, :], outi_t[:])

    nc.compile()
    return nc


def build_nc_f16x3(n_rows: int):
    """Phase-2 exact rescore, fp16 hi/lo 3-term (error ~3e-6 on randn data).

    Same pre-transposed tile layout as the f16 screen, but x ships as
    [xh; xl] (fp16 value + fp16 residual) and c as ch + cl constants.
    s = xh.ch + xh.cl + xl.ch (+bias), argmax via f32-PSUM max/max_index
    (fp16 reduction would blur the tight margins phase-2 rows have).
    """
    assert n_rows % P == 0
    nt = n_rows // P
    nc = bacc.Bacc("TRN2", target_bir_lowering=False, debug=False)

    F16 = mybir.dt.float16
    U16 = mybir.dt.uint16

    xt = nc.dram_tensor("xt", [nt, P, 2 * D], F16, kind="ExternalInput").ap()
    ch_in = nc.dram_tensor("ch", [D, K], F16, kind="ExternalInput").ap()
    cl_in = nc.dram_tensor("cl", [D, K], F16, kind="ExternalInput").ap()
    bias2 = nc.dram_tensor("bias2", [2, K], F16, kind="ExternalInput").ap()
    ones = nc.dram_tensor("ones", [2, P], F16, kind="ExternalInput").ap()
    outi = nc.dram_tensor("outi", [P, nt * 8], U16, kind="ExternalOutput").ap()

    with tile.TileContext(nc) as tc, ExitStack() as ctx:
        const = ctx.enter_context(tc.tile_pool(name="const", bufs=1))
        xin_p = ctx.enter_context(tc.tile_pool(name="xin", bufs=3))
        sc_p = ctx.enter_context(tc.tile_pool(name="sc", bufs=2, space="PSUM"))
        mx_p = ctx.enter_context(tc.tile_pool(name="mx", bufs=2))
        ob_p = ctx.enter_context(tc.tile_pool(name="ob", bufs=1))

        bias_t = const.tile([2, K], F16, tag="bias2", name="bias2")
        nc.sync.dma_start(bias_t[:], bias2[:, :])
        ones_t = const.tile([2, P], F16, tag="ones", name="ones")
        nc.sync.dma_start(ones_t[:], ones[:, :])
        c_tiles = {}
        for nm, src in (("ch", ch_in), ("cl", cl_in)):
            c3 = src.rearrange("(t p) k -> t p k", p=P)
            for d in range(DT):
                ct = const.tile([P, K], F16, tag=f"{nm}_{d}", name=f"{nm}_{d}")
                nc.sync.dma_start(ct[:], c3[d])
                c_tiles[(nm, d)] = ct

        outi_t = ob_p.tile([P, nt * 8], U16, tag="outi", name="outi_t")

        for t in range(nt):
            xts = xin_p.tile([P, 2 * D], F16, name="xts")
            nc.gpsimd.dma_start(xts[:], xt[t])

            def w(h, d):  # stationary x chunk: h=0 hi, h=1 lo
                off = h * D + d * P
                return xts[:, off:off + P]

            ss = sc_p.tile([P, K], F32, name="ss")
            for b in range(NB):
                nc.tensor.matmul(
                    ss[:, KOFF[b]:KOFF[b] + KW[b]], ones_t[:],
                    bias_t[:, KOFF[b]:KOFF[b] + KW[b]],
                    start=True, stop=False, skip_group_check=True)
            terms = [(0, "ch"), (0, "cl"), (1, "ch")]
            for ti, (h, cn) in enumerate(terms):
                for d in range(DT):
                    last = ti == len(terms) - 1 and d == DT - 1
                    for b in range(NB):
                        nc.tensor.matmul(
                            ss[:, KOFF[b]:KOFF[b] + KW[b]], w(h, d),
                            c_tiles[(cn, d)][:, KOFF[b]:KOFF[b] + KW[b]],
                            start=False, stop=last, skip_group_check=True)

            mxv = mx_p.tile([P, 8], F32, name="mxv")
            nc.vector.max(mxv[:], ss[:])
            nc.vector.max_index(outi_t[:, t * 8:(t + 1) * 8], mxv[:], ss[:])

        nc.sync.dma_start(outi[:, :], outi_t[:])

    nc.compile()
    return nc


def make_f16x3_in_maps(x_rows: np.ndarray, centroids: np.ndarray,
                       n_rows: int, n_cores: int = NCORES):
    """x_rows: gathered flagged rows [m, D] f32, padded to n_rows*n_cores."""
    c = np.ascontiguousarray(centroids, dtype=np.float32)
    nt = n_rows // P
    ch = c.astype(np.float16)
    cl = (c - ch.astype(np.float32)).astype(np.float16)
    c_norm = (c.astype(np.float64) ** 2).sum(axis=0)
    b = (-0.5 * c_norm).astype(np.float32)
    b_hi = b.astype(np.float16)
    b_lo = (b - b_hi.astype(np.float32)).astype(np.float16)
    base = {
        "ch": ch, "cl": cl,
        "bias2": np.stack([b_hi, b_lo]),
        "ones": np.ones((2, P), dtype=np.float16),
    }
    total = n_rows * n_cores
    xp = np.zeros((total, D), dtype=np.float32)
    xp[: len(x_rows)] = x_rows
    xh = xp.astype(np.float16)
    xl = (xp - xh.astype(np.float32)).astype(np.float16)
    in_maps = []
    for i in range(n_cores):
        m = dict(base)
        parts = []
        for arr in (xh, xl):
            ac = arr[i * n_rows:(i + 1) * n_rows]
            parts.append(np.ascontiguousarray(
                ac.reshape(nt, P, DT, P).transpose(0, 3, 2, 1)).reshape(nt, P, D))
        m["xt"] = np.concatenate(parts, axis=2)  # [nt, P, 2D]
        in_maps.append(m)
    return in_maps


def _run_exact_f16(x_rows: np.ndarray, centroids: np.ndarray, n_rows: int):
    nc = _cached_nc(("f16x3", n_rows), lambda: build_nc_f16x3(n_rows))
    in_maps = make_f16x3_in_maps(x_rows, centroids, n_rows)
    res = _run_spmd(nc, in_maps, "phase2")
    nt = n_rows // P
    parts = []
    for i in range(NCORES):
        ii = res.results[i]["outi"].reshape(P, nt, 8)
        parts.append(ii.transpose(1, 0, 2).reshape(n_rows, 8)[:, 0])
    out = np.concatenate(parts).astype(np.int32)
    return out[: len(x_rows)], res


def make_f16_in_maps(x: np.ndarray, centroids: np.ndarray,
                     n_rows: int = NSH, n_cores: int = NCORES):
    x = np.ascontiguousarray(x, dtype=np.float32)
    c = np.ascontiguousarray(centroids, dtype=np.float32)
    nt = n_rows // P
    c_norm = (c.astype(np.float64) ** 2).sum(axis=0)
    b = (384.0 - 0.5 * c_norm).astype(np.float32)  # pre-shifted bias
    b_hi = b.astype(np.float16)
    b_lo = (b - b_hi.astype(np.float32)).astype(np.float16)
    base = {
        "c": c.astype(np.float16),
        "bias2": np.stack([b_hi, b_lo]),
        "biasr": np.broadcast_to(b_hi, (P, K)).copy(),
        "ones": np.ones((2, P), dtype=np.float16),
    }
    x16 = x.astype(np.float16)
    in_maps = []
    for i in range(n_cores):
        xc = x16[i * n_rows:(i + 1) * n_rows]
        # xt[t, d, c, col] = x[t*128+col, c*128+d]
        xt = np.ascontiguousarray(
            xc.reshape(nt, P, DT, P).transpose(0, 3, 2, 1)).reshape(nt, P, D)
        m = dict(base)
        m["xt"] = xt
        in_maps.append(m)
    return in_maps


def build_nc(mode: str = "bf16x3", n_rows: int = NSH):
    """Build + compile the per-core Bass program.

    mode: 'bf16x3' (hi/lo split, 3 bf16 passes), 'fp32', 'fp32r', 'bf16'
    """
    assert n_rows % P == 0
    nt = n_rows // P
    nc = bacc.Bacc("TRN2", target_bir_lowering=False, debug=False)

    x = nc.dram_tensor("x", [n_rows, D], F32, kind="ExternalInput").ap()
    bias = nc.dram_tensor("bias", [P, K], F32, kind="ExternalInput").ap()
    out = nc.dram_tensor("out", [n_rows, 1], U32, kind="ExternalOutput").ap()

    split = mode == "bf16x3"
    cdt = BF if mode in ("bf16x3", "bf16") else F32
    mmdt = {"bf16x3": BF, "bf16": BF, "fp32": F32, "fp32r": mybir.dt.float32r}[mode]

    if split:
        c_hi = nc.dram_tensor("c_hi", [D, K], BF, kind="ExternalInput").ap()
        c_lo = nc.dram_tensor("c_lo", [D, K], BF, kind="ExternalInput").ap()
        c_srcs = [c_hi, c_lo]
    else:
        c_full = nc.dram_tensor("c", [D, K], cdt, kind="ExternalInput").ap()
        c_srcs = [c_full]
    ident = nc.dram_tensor("ident", [P, P], mmdt if mmdt != mybir.dt.float32r else F32,
                           kind="ExternalInput").ap()

    with tile.TileContext(nc) as tc, ExitStack() as ctx:
        const = ctx.enter_context(tc.tile_pool(name="const", bufs=1))
        xin_p = ctx.enter_context(tc.tile_pool(name="xin", bufs=3))
        xcast_p = ctx.enter_context(tc.tile_pool(name="xcast", bufs=2))
        xtp_p = ctx.enter_context(tc.tile_pool(name="xtp", bufs=2, space="PSUM"))
        xts_p = ctx.enter_context(tc.tile_pool(name="xts", bufs=2))
        sc_p = ctx.enter_context(tc.tile_pool(name="sc", bufs=1, space="PSUM"))
        ss_p = ctx.enter_context(tc.tile_pool(name="ss", bufs=2))
        mx_p = ctx.enter_context(tc.tile_pool(name="mx", bufs=4))

        # centroids resident in SBUF: [DT][P, K] per source (hi/lo or single)
        c_tiles = []
        for si, csrc in enumerate(c_srcs):
            c3 = csrc.rearrange("(t p) k -> t p k", p=P)
            tiles = []
            for d in range(DT):
                ct = const.tile([P, K], cdt, tag=f"c{si}_{d}")
                nc.sync.dma_start(ct[:], c3[d])
                tiles.append(ct)
            c_tiles.append(tiles)

        bias_t = const.tile([P, K], F32, tag="bias")
        nc.sync.dma_start(bias_t[:], bias[:, :])
        id_t = const.tile([P, P], ident.dtype, tag="ident")
        nc.sync.dma_start(id_t[:], ident[:, :])

        for t in range(nt):
            xin = xin_p.tile([P, D], F32)
            nc.scalar.dma_start(xin[:], x[t * P:(t + 1) * P, :])

            if split:
                xh = xcast_p.tile([P, D], BF, tag="xh")
                nc.scalar.copy(xh[:], xin[:])
                xl = xcast_p.tile([P, D], BF, tag="xl")
                nc.vector.tensor_sub(xl[:], xin[:], xh[:])
                tsrc = [xh, xl]
            elif mode == "bf16":
                xh = xcast_p.tile([P, D], BF, tag="xh")
                nc.scalar.copy(xh[:], xin[:])
                tsrc = [xh]
            else:
                tsrc = [xin]

            # transpose x tiles -> [d, n] layout for matmul weights
            nsrc = len(tsrc)
            tdt = BF if cdt == BF else F32
            xtp = xtp_p.tile([P, D * nsrc], tdt)
            for si, xsrc in enumerate(tsrc):
                for d in range(DT):
                    nc.tensor.transpose(
                        xtp[:, si * D + d * P: si * D + (d + 1) * P],
                        xsrc[:, d * P:(d + 1) * P],
                        id_t[:],
                    )
            xts = xts_p.tile([P, D * nsrc], tdt)
            nc.scalar.copy(xts[:], xtp[:])

            def w(si, d):
                return xts[:, si * D + d * P: si * D + (d + 1) * P]

            banks = []
            for b in range(NB):
                bank_tile = sc_p.tile([P, KW[b]], F32, tag=f"b{b}", name=f"bank{b}")
                banks.append(bank_tile)
            if split:
                # accumulate xh.ch + xh.cl + xl.ch over d
                for d in range(DT):
                    for b in range(NB):
                        nc.tensor.matmul(
                            banks[b][:], w(0, d),
                            c_tiles[0][d][:, KOFF[b]:KOFF[b] + KW[b]],
                            start=(d == 0), stop=False)
                    for b in range(NB):
                        nc.tensor.matmul(
                            banks[b][:], w(0, d),
                            c_tiles[1][d][:, KOFF[b]:KOFF[b] + KW[b]],
                            start=False, stop=False)
                    for b in range(NB):
                        nc.tensor.matmul(
                            banks[b][:], w(1, d),
                            c_tiles[0][d][:, KOFF[b]:KOFF[b] + KW[b]],
                            start=False, stop=(d == DT - 1))
            else:
                for d in range(DT):
                    for b in range(NB):
                        lhs = w(0, d)
                        rhs = c_tiles[0][d][:, KOFF[b]:KOFF[b] + KW[b]]
                        if mode == "fp32r":
                            lhs = lhs.bitcast(mybir.dt.float32r)
                            rhs = rhs.bitcast(mybir.dt.float32r)
                        nc.tensor.matmul(banks[b][:], lhs, rhs,
                                         start=(d == 0), stop=(d == DT - 1))

            ss = ss_p.tile([P, K], F32)
            for b in range(NB):
                nc.vector.tensor_add(
                    ss[:, KOFF[b]:KOFF[b] + KW[b]], banks[b][:],
                    bias_t[:, KOFF[b]:KOFF[b] + KW[b]])

            mxv = mx_p.tile([P, 8], F32, tag="mxv")
            nc.vector.max(mxv[:], ss[:])
            mxi = mx_p.tile([P, 8], U32, tag="mxi")
            nc.vector.max_index(mxi[:], mxv[:], ss[:])
            nc.scalar.dma_start(out[t * P:(t + 1) * P, :], mxi[:, 0:1])

    nc.compile()
    return nc


def make_in_maps(x: np.ndarray, centroids: np.ndarray, mode: str = "bf16x3",
                 n_rows: int = NSH, n_cores: int = NCORES):
    x = np.ascontiguousarray(x, dtype=np.float32)
    c = np.ascontiguousarray(centroids, dtype=np.float32)
    c_norm = (c.astype(np.float64) ** 2).sum(axis=0)
    bias = np.broadcast_to((-0.5 * c_norm).astype(np.float32), (P, K)).copy()

    base = {"bias": bias}
    if mode == "bf16x3":
        c_hi = c.astype(BF16)
        c_lo = (c - c_hi.astype(np.float32)).astype(BF16)
        base["c_hi"] = c_hi
        base["c_lo"] = c_lo
        base["ident"] = np.eye(P, dtype=BF16)
    elif mode == "bf16":
        base["c"] = c.astype(BF16)
        base["ident"] = np.eye(P, dtype=BF16)
    else:
        base["c"] = c
        base["ident"] = np.eye(P, dtype=np.float32)

    in_maps = []
    for i in range(n_cores):
        m = dict(base)
        m["x"] = x[i * n_rows:(i + 1) * n_rows]
        in_maps.append(m)
    return in_maps


_NC_CACHE = {}
LAST_RESULTS = []  # (label, BassKernelResults) of the most recent kernel() call


def _run_spmd(nc, in_maps, label):
    kw = {}
    if os.environ.get("KMEANS_TRACE"):
        import shutil

        kw["trace"] = True
        kw["tmpdir"] = os.environ.get("KMEANS_TRACE_DIR", "/tmp/km_trace") + "_" + label
        shutil.rmtree(kw["tmpdir"], ignore_errors=True)
        os.makedirs(kw["tmpdir"], exist_ok=True)
    res = run_bass_kernel_spmd(nc, in_maps, core_ids=list(range(NCORES)), **kw)
    LAST_RESULTS.append((label, res))
    return res

# Phase-2 capacity: rows per core recomputed exactly. Margin threshold:
# empirical max bf16 score error on randn data is ~0.2; flag anything under
# 4x that. ~5% of rows get flagged at this threshold.
P2_ROWS = 1024
MARGIN_TH = None  # set below after calibration constant


def _cached_nc(key, builder):
    if key not in _NC_CACHE:
        _NC_CACHE[key] = builder()
    return _NC_CACHE[key]


def make_screen_in_maps(x: np.ndarray, centroids: np.ndarray,
                        n_rows: int = NSH, n_cores: int = NCORES):
    x = np.ascontiguousarray(x, dtype=np.float32)
    c = np.ascontiguousarray(centroids, dtype=np.float32)
    c_norm = (c.astype(np.float64) ** 2).sum(axis=0)
    bias = (-0.5 * c_norm).astype(np.float32)
    bias_hi = bias.astype(BF16)
    bias_lo = (bias - bias_hi.astype(np.float32)).astype(BF16)
    base = {
        "c": c.astype(BF16),
        "bias2": np.stack([bias_hi, bias_lo]),
        "ones": np.ones((2, P), dtype=BF16),
        "ident": np.eye(P, dtype=BF16),
    }
    in_maps = []
    for i in range(n_cores):
        m = dict(base)
        m["x"] = x[i * n_rows:(i + 1) * n_rows]
        in_maps.append(m)
    return in_maps


def _run_exact(x_rows: np.ndarray, centroids: np.ndarray, n_rows: int):
    """Run the exact (bf16x3) program on x_rows padded to n_rows*NCORES."""
    nc = _cached_nc(("bf16x3", n_rows), lambda: build_nc("bf16x3", n_rows))
    total = n_rows * NCORES
    xp = np.zeros((total, D), dtype=np.float32)
    xp[: len(x_rows)] = x_rows
    in_maps = make_in_maps(xp, centroids, mode="bf16x3", n_rows=n_rows)
    res = _run_spmd(nc, in_maps, "phase2")
    out = np.concatenate(
        [res.results[i]["out"].reshape(n_rows) for i in range(NCORES)])
    return out[: len(x_rows)], res


def kernel(x: np.ndarray, centroids: np.ndarray) -> np.ndarray:
    mode = os.environ.get("KMEANS_MODE", "f16")
    LAST_RESULTS.clear()
    x = np.asarray(x)
    centroids = np.asarray(centroids)

    if mode == "f16":
        # phase 1: fp16 screen with top-8 values + indices
        nc1 = _cached_nc(("f16", NSH), lambda: build_nc_f16(NSH))
        in_maps = make_f16_in_maps(x, centroids)
        res1 = _run_spmd(nc1, in_maps, "phase1")
        nt = NSH // P
        idx_parts, val_parts = [], []
        for i in range(NCORES):
            vi = res1.results[i]["outv"].reshape(P, nt, 8).astype(np.float32)
            ii = res1.results[i]["outi"].reshape(P, nt, 8)
            # row n = t*128 + p  ->  [t, p]
            val_parts.append(vi.transpose(1, 0, 2).reshape(NSH, 8))
            idx_parts.append(ii.transpose(1, 0, 2).reshape(NSH, 8))
        vals = np.concatenate(val_parts)
        idx = np.concatenate(idx_parts)[:, 0].astype(np.int32)

        # fp16 values: quantization ~0.06-0.125 near |s'|<=256 plus fp16
        # matmul error ~0.05 -> th=0.3 flags everything at risk
        margin = vals[:, 0] - vals[:, 1]
        th = float(os.environ.get("KMEANS_MARGIN_TH", "0.3"))
        flagged = np.flatnonzero(margin < th)

        # cap at 4096 (phase-2's 512-rows/core tier): if slightly more are
        # flagged, drop the widest margins -- those are far above the fp16
        # screen's worst-case error, so they cannot actually be mis-picks
        CAP = 4096
        if len(flagged) > CAP:
            keep = np.argpartition(margin[flagged], CAP - 1)[:CAP]
            flagged = flagged[keep]
        if os.environ.get("KMEANS_DEBUG"):
            print(f"[f16] flagged={len(flagged)} ({100 * len(flagged) / N:.2f}%)",
                  flush=True)
        sizes = [128, 256, 512, 768, 1024, 2048]
        per_core = min((s for s in sizes if s * NCORES >= len(flagged)),
                       default=sizes[-1])
        cap = per_core * NCORES
        for s in range(0, len(flagged), cap):
            rows = flagged[s:s + cap]
            exact_idx, _ = _run_exact_f16(x[rows], centroids, per_core)
            idx[rows] = exact_idx
        return idx

    if mode != "hybrid":
        nc = _cached_nc((mode, NSH), lambda: build_nc(mode=mode))
        in_maps = make_in_maps(x, centroids, mode=mode)
        res = _run_spmd(nc, in_maps, mode)
        parts = [res.results[i]["out"].reshape(NSH) for i in range(NCORES)]
        return np.concatenate(parts).astype(np.int32)

    # phase 1: bf16 screen with top-2 margins
    nc1 = _cached_nc(("screen", NSH), lambda: build_nc_screen(NSH))
    in_maps = make_screen_in_maps(x, centroids)
    res1 = _run_spmd(nc1, in_maps, "phase1")
    idx = np.concatenate(
        [res1.results[i]["out"].reshape(NSH) for i in range(NCORES)]
    ).astype(np.int32)
    vals = np.concatenate(
        [res1.results[i]["vals"].reshape(NSH, 2) for i in range(NCORES)])

    margin = vals[:, 0] - vals[:, 1]
    th = float(os.environ.get("KMEANS_MARGIN_TH", "0.8"))
    flagged = np.flatnonzero(margin < th)

    # phase 2: exact recompute of flagged rows; pick the smallest padded
    # program that covers the count, chunking in the (unexpected) overflow case
    sizes = [512, 1024, 1536, 2048]
    per_core = min((s for s in sizes if s * NCORES >= len(flagged)),
                   default=sizes[-1])
    cap = per_core * NCORES
    for s in range(0, len(flagged), cap):
        rows = flagged[s:s + cap]
        exact_idx, _ = _run_exact(x[rows], centroids, per_core)
        idx[rows] = exact_idx
    return idx



# revision 15
# speedup vs baseline: 1.8065x; 1.0063x over previous
"""Trainium2 Bass kernel for KMeans assignment (argmin over centroid distances).

Problem: x [131072, 768] f32, centroids [768, 2000] f32
Output:  argmin_k ||x_n - c_k||^2  -> int32 [131072]

Math: argmin_k(||x||^2 - 2 x.c_k + ||c_k||^2) = argmax_k(x.c_k - 0.5||c_k||^2).
Data-parallel over 8 cores (16384 rows each); two phases:

Phase 1 (f16 screen, ~734us): host pre-transposes + pre-casts x to fp16 in
tile layout (no on-device cast/transpose); fp16 centroids resident in SBUF.
Per 128-row tile: 24 matmuls accumulate scores into a 4-bank f32 PSUM tile
(plus one 2-row ones-matmul adding the pre-shifted bias b+384 to bank 0);
the scalar engine copies PSUM to a recentered fp16 SBUF tile; the DVE adds
the fp16 bias residual on banks 1-3 (2-byte tensor_tensor = 2x rate), then
MAX8 + FIND_INDEX8 on fp16 (f32/PSUM max pairs are ~2x slower due to pipe
drains). Top-8 values/indices batch in SBUF, one DMA at the end, so the PE
streams back-to-back (~5.5us/tile = mm floor; no HAM re-throttle).

Phase 2 (~97us): rows whose fp16 top-2 margin < 0.3 (capped to the 4096
smallest margins; ~3%) are rescored exactly (fp16 hi/lo 3-term, error
~3e-6) with argmax via f32-PSUM max/max_index.
"""

import os
import sys

for _p in ("/opt/trn_rl_repo",):
    if _p not in sys.path and os.path.isdir(_p):
        sys.path.insert(0, _p)

from contextlib import ExitStack

import numpy as np

import concourse.bass as bass
import concourse.tile as tile
from concourse import bacc, mybir
from concourse.bass_utils import run_bass_kernel_spmd

try:
    import ml_dtypes

    BF16 = np.dtype(ml_dtypes.bfloat16)
except ImportError:  # pragma: no cover
    BF16 = None

N, D, K = 131072, 768, 2000
NCORES = 8
NSH = N // NCORES  # 16384 rows per core
P = 128
DT = D // P  # 6 contraction tiles
# score chunks, each within one PSUM bank (<=512 fp32)
KOFF = [0, 512, 1024, 1536]
KW = [512, 512, 512, 464]
NB = 4

F32 = mybir.dt.float32
BF = mybir.dt.bfloat16
U32 = mybir.dt.uint32


def build_nc_screen(n_rows: int = NSH):
    """Phase-1 screening program: single bf16 matmul pass.

    Bias (-0.5||c||^2) is folded into the matmul as two extra contraction
    rows (ones-weights x [bias_hi; bias_lo]) so the vector engine only runs
    max/max_index. Outputs the argmax index and the top-2 score values per
    row; rows with a small top-2 margin get recomputed exactly in phase 2.
    """
    assert n_rows % P == 0
    nt = n_rows // P
    nc = bacc.Bacc("TRN2", target_bir_lowering=False, debug=False)

    x = nc.dram_tensor("x", [n_rows, D], F32, kind="ExternalInput").ap()
    c_in = nc.dram_tensor("c", [D, K], BF, kind="ExternalInput").ap()
    bias2 = nc.dram_tensor("bias2", [2, K], BF, kind="ExternalInput").ap()
    ones = nc.dram_tensor("ones", [2, P], BF, kind="ExternalInput").ap()
    ident = nc.dram_tensor("ident", [P, P], BF, kind="ExternalInput").ap()
    out = nc.dram_tensor("out", [n_rows, 1], U32, kind="ExternalOutput").ap()
    vals = nc.dram_tensor("vals", [n_rows, 2], F32, kind="ExternalOutput").ap()

    with tile.TileContext(nc) as tc, ExitStack() as ctx:
        const = ctx.enter_context(tc.tile_pool(name="const", bufs=1))
        xin_p = ctx.enter_context(tc.tile_pool(name="xin", bufs=3))
        xcast_p = ctx.enter_context(tc.tile_pool(name="xcast", bufs=2))
        xtp_p = ctx.enter_context(tc.tile_pool(name="xtp", bufs=1, space="PSUM"))
        xts_p = ctx.enter_context(tc.tile_pool(name="xts", bufs=2))
        sc_p = ctx.enter_context(tc.tile_pool(name="sc", bufs=1, space="PSUM"))
        ss_p = ctx.enter_context(tc.tile_pool(name="ss", bufs=2))
        mx_p = ctx.enter_context(tc.tile_pool(name="mx", bufs=4))

        c3 = c_in.rearrange("(t p) k -> t p k", p=P)
        c_tiles = []
        for d in range(DT):
            ct = const.tile([P, K], BF, tag=f"c_{d}")
            nc.sync.dma_start(ct[:], c3[d])
            c_tiles.append(ct)
        bias_t = const.tile([2, K], BF, tag="bias2")
        nc.sync.dma_start(bias_t[:], bias2[:, :])
        ones_t = const.tile([2, P], BF, tag="ones")
        nc.sync.dma_start(ones_t[:], ones[:, :])
        id_t = const.tile([P, P], BF, tag="ident")
        nc.sync.dma_start(id_t[:], ident[:, :])

        for t in range(nt):
            xin = xin_p.tile([P, D], F32)
            nc.scalar.dma_start(xin[:], x[t * P:(t + 1) * P, :])
            xh = xcast_p.tile([P, D], BF, tag="xh")
            nc.scalar.copy(xh[:], xin[:])
            xtp = xtp_p.tile([P, D], BF)
            for d in range(DT):
                nc.tensor.transpose(
                    xtp[:, d * P:(d + 1) * P], xh[:, d * P:(d + 1) * P], id_t[:])
            xts = xts_p.tile([P, D], BF)
            nc.scalar.copy(xts[:], xtp[:])

            banks = []
            for b in range(NB):
                # first three banks double-buffered: they are what the next
                # tile's first matmuls wait on (PSUM budget: 2+2+2+1 + 1 xtp)
                bank_tile = sc_p.tile([P, KW[b]], F32, tag=f"b{b}",
                                      name=f"bank{b}", bufs=(2 if b < 3 else 1))
                banks.append(bank_tile)
            # bias rows first so each bank's accumulation closes on d == DT-1
            for b in range(NB):
                nc.tensor.matmul(
                    banks[b][:], ones_t[:],
                    bias_t[:, KOFF[b]:KOFF[b] + KW[b]],
                    start=True, stop=False)
            for d in range(DT):
                for b in range(NB):
                    nc.tensor.matmul(
                        banks[b][:], xts[:, d * P:(d + 1) * P],
                        c_tiles[d][:, KOFF[b]:KOFF[b] + KW[b]],
                        start=False, stop=(d == DT - 1))

            ss = ss_p.tile([P, K], F32)
            for b in range(NB):
                nc.scalar.copy(ss[:, KOFF[b]:KOFF[b] + KW[b]], banks[b][:])

            mxv = mx_p.tile([P, 8], F32, tag="mxv")
            nc.vector.max(mxv[:], ss[:])
            mxi = mx_p.tile([P, 8], U32, tag="mxi")
            nc.vector.max_index(mxi[:], mxv[:], ss[:])
            nc.scalar.dma_start(out[t * P:(t + 1) * P, :], mxi[:, 0:1])
            nc.scalar.dma_start(vals[t * P:(t + 1) * P, :], mxv[:, 0:2])

    nc.compile()
    return nc


def build_nc_f16(n_rows: int = NSH):
    """fp16 screening program, v2 of the screen.

    Differences from build_nc_screen (bf16):
      - x arrives from the host pre-transposed AND pre-cast to fp16 in tile
        layout xt[t, d, c, col] = x[t*128+col, c*128+d]: no on-device cast,
        no PE transpose, no PSUM->SBUF staging copy. fp16 mm runs at bf16
        speed with ~8x less quantization error -> far fewer phase-2 rows.
      - scores accumulate into ONE 4-bank PSUM tile [128, 2000]; the scalar
        engine copies it to fp16 SBUF (scores pre-shifted by +384 via the
        bias constants so fp16 ulp stays small), and the DVE runs the
        max/max_index pair on fp16 (f32/PSUM pairs are ~2x slower on HW).
      - outputs (top-8 fp16 values + u16 indices) collect in SBUF and ship
        as two DMAs at the end: nothing downstream blocks on DVE results,
        so the PE never idles (avoids HAM re-throttle to 1.2 GHz).
      - x DMA issued from the otherwise-idle gpsimd sequencer.
    """
    assert n_rows % P == 0
    nt = n_rows // P
    nc = bacc.Bacc("TRN2", target_bir_lowering=False, debug=False)

    F16 = mybir.dt.float16
    U16 = mybir.dt.uint16

    # bias2/biasr hold hi/lo resp. fp16 of (b + 384): the +384 recenters
    # scores near 0 so every fp16 intermediate keeps small ulp
    xt = nc.dram_tensor("xt", [nt, P, D], F16, kind="ExternalInput").ap()
    c_in = nc.dram_tensor("c", [D, K], F16, kind="ExternalInput").ap()
    bias2 = nc.dram_tensor("bias2", [2, K], F16, kind="ExternalInput").ap()
    biasr = nc.dram_tensor("biasr", [P, K], F16, kind="ExternalInput").ap()
    ones = nc.dram_tensor("ones", [2, P], F16, kind="ExternalInput").ap()
    outv = nc.dram_tensor("outv", [P, nt * 8], F16, kind="ExternalOutput").ap()
    outi = nc.dram_tensor("outi", [P, nt * 8], U16, kind="ExternalOutput").ap()

    with tile.TileContext(nc) as tc, ExitStack() as ctx:
        const = ctx.enter_context(tc.tile_pool(name="const", bufs=1))
        xin_p = ctx.enter_context(tc.tile_pool(name="xin", bufs=4))
        sc_p = ctx.enter_context(tc.tile_pool(name="sc", bufs=2, space="PSUM"))
        sf_p = ctx.enter_context(tc.tile_pool(name="sf", bufs=2))
        ob_p = ctx.enter_context(tc.tile_pool(name="ob", bufs=1))

        # small consts first so tile 0's bias matmul can start immediately
        bias_t = const.tile([2, K], F16, tag="bias2", name="bias2")
        nc.sync.dma_start(bias_t[:], bias2[:, :])
        ones_t = const.tile([2, P], F16, tag="ones", name="ones")
        nc.sync.dma_start(ones_t[:], ones[:, :])
        c3 = c_in.rearrange("(t p) k -> t p k", p=P)
        c_tiles = []
        for d in range(DT):
            ct = const.tile([P, K], F16, tag=f"c_{d}", name=f"c_{d}")
            nc.sync.dma_start(ct[:], c3[d])
            c_tiles.append(ct)
        biasr_t = const.tile([P, K], F16, tag="biasr", name="biasr")
        nc.sync.dma_start(biasr_t[:], biasr[:, :])

        outv_t = ob_p.tile([P, nt * 8], F16, tag="outv", name="outv_t")
        outi_t = ob_p.tile([P, nt * 8], U16, tag="outi", name="outi_t")

        for t in range(nt):
            xts = xin_p.tile([P, D], F16, name="xts")
            nc.gpsimd.dma_start(xts[:], xt[t])

            ss = sc_p.tile([P, K], F32, name="ss")
            # bias via PE ones-matmul for bank 0 only; banks 1-3 get the
            # (b+384) residual added by the DVE on the fp16 copy (2x rate),
            # balancing PE (~5.43us) against DVE (~5.3us) per tile
            nc.tensor.matmul(
                ss[:, 0:KW[0]], ones_t[:], bias_t[:, 0:KW[0]],
                start=True, stop=False, skip_group_check=True)
            for d in range(DT):
                for b in range(NB):
                    nc.tensor.matmul(
                        ss[:, KOFF[b]:KOFF[b] + KW[b]],
                        xts[:, d * P:(d + 1) * P],
                        c_tiles[d][:, KOFF[b]:KOFF[b] + KW[b]],
                        start=(d == 0 and b > 0), stop=(d == DT - 1),
                        skip_group_check=True)

            # fp16 copy: fp16 max/max_index pairs run ~2x faster than
            # f32/PSUM ones (no inter-op pipe-drain bubbles)
            sf = sf_p.tile([P, K], F16, name="sf")
            nc.scalar.copy(sf[:], ss[:])
            nc.vector.tensor_add(sf[:, KW[0]:K], sf[:, KW[0]:K],
                                 biasr_t[:, KW[0]:K])

            vslot = outv_t[:, t * 8:(t + 1) * 8]
            nc.vector.max(vslot, sf[:])
            nc.vector.max_index(outi_t[:, t * 8:(t + 1) * 8], vslot, sf[:])

        nc.sync.dma_start(outv[:, :], outv_t[:])
        nc.sync.dma_start(outi[:, :], outi_t[:])

    nc.compile()
    return nc


def build_nc_f16x3(n_rows: int):
    """Phase-2 exact rescore, fp16 hi/lo 3-term (error ~3e-6 on randn data).

    Same pre-transposed tile layout as the f16 screen, but x ships as
    [xh; xl] (fp16 value + fp16 residual) and c as ch + cl constants.
    s = xh.ch + xh.cl + xl.ch (+bias), argmax via f32-PSUM max/max_index
    (fp16 reduction would blur the tight margins phase-2 rows have).
    """
    assert n_rows % P == 0
    nt = n_rows // P
    nc = bacc.Bacc("TRN2", target_bir_lowering=False, debug=False)

    F16 = mybir.dt.float16
    U16 = mybir.dt.uint16

    xt = nc.dram_tensor("xt", [nt, P, 2 * D], F16, kind="ExternalInput").ap()
    ch_in = nc.dram_tensor("ch", [D, K], F16, kind="ExternalInput").ap()
    cl_in = nc.dram_tensor("cl", [D, K], F16, kind="ExternalInput").ap()
    bias2 = nc.dram_tensor("bias2", [2, K], F16, kind="ExternalInput").ap()
    ones = nc.dram_tensor("ones", [2, P], F16, kind="ExternalInput").ap()
    outi = nc.dram_tensor("outi", [P, nt * 8], U16, kind="ExternalOutput").ap()

    with tile.TileContext(nc) as tc, ExitStack() as ctx:
        const = ctx.enter_context(tc.tile_pool(name="const", bufs=1))
        xin_p = ctx.enter_context(tc.tile_pool(name="xin", bufs=3))
        sc_p = ctx.enter_context(tc.tile_pool(name="sc", bufs=2, space="PSUM"))
        mx_p = ctx.enter_context(tc.tile_pool(name="mx", bufs=2))
        ob_p = ctx.enter_context(tc.tile_pool(name="ob", bufs=1))

        bias_t = const.tile([2, K], F16, tag="bias2", name="bias2")
        nc.sync.dma_start(bias_t[:], bias2[:, :])
        ones_t = const.tile([2, P], F16, tag="ones", name="ones")
        nc.sync.dma_start(ones_t[:], ones[:, :])
        c_tiles = {}
        for nm, src in (("ch", ch_in), ("cl", cl_in)):
            c3 = src.rearrange("(t p) k -> t p k", p=P)
            for d in range(DT):
                ct = const.tile([P, K], F16, tag=f"{nm}_{d}", name=f"{nm}_{d}")
                nc.sync.dma_start(ct[:], c3[d])
                c_tiles[(nm, d)] = ct

        outi_t = ob_p.tile([P, nt * 8], U16, tag="outi", name="outi_t")

        for t in range(nt):
            xts = xin_p.tile([P, 2 * D], F16, name="xts")
            nc.gpsimd.dma_start(xts[:], xt[t])

            def w(h, d):  # stationary x chunk: h=0 hi, h=1 lo
                off = h * D + d * P
                return xts[:, off:off + P]

            ss = sc_p.tile([P, K], F32, name="ss")
            for b in range(NB):
                nc.tensor.matmul(
                    ss[:, KOFF[b]:KOFF[b] + KW[b]], ones_t[:],
                    bias_t[:, KOFF[b]:KOFF[b] + KW[b]],
                    start=True, stop=False, skip_group_check=True)
            terms = [(0, "ch"), (0, "cl"), (1, "ch")]
            for ti, (h, cn) in enumerate(terms):
                for d in range(DT):
                    last = ti == len(terms) - 1 and d == DT - 1
                    for b in range(NB):
                        nc.tensor.matmul(
                            ss[:, KOFF[b]:KOFF[b] + KW[b]], w(h, d),
                            c_tiles[(cn, d)][:, KOFF[b]:KOFF[b] + KW[b]],
                            start=False, stop=last, skip_group_check=True)

            mxv = mx_p.tile([P, 8], F32, name="mxv")
            nc.vector.max(mxv[:], ss[:])
            nc.vector.max_index(outi_t[:, t * 8:(t + 1) * 8], mxv[:], ss[:])

        nc.sync.dma_start(outi[:, :], outi_t[:])

    nc.compile()
    return nc


def make_f16x3_in_maps(x_rows: np.ndarray, centroids: np.ndarray,
                       n_rows: int, n_cores: int = NCORES):
    """x_rows: gathered flagged rows [m, D] f32, padded to n_rows*n_cores."""
    c = np.ascontiguousarray(centroids, dtype=np.float32)
    nt = n_rows // P
    ch = c.astype(np.float16)
    cl = (c - ch.astype(np.float32)).astype(np.float16)
    c_norm = (c.astype(np.float64) ** 2).sum(axis=0)
    b = (-0.5 * c_norm).astype(np.float32)
    b_hi = b.astype(np.float16)
    b_lo = (b - b_hi.astype(np.float32)).astype(np.float16)
    base = {
        "ch": ch, "cl": cl,
        "bias2": np.stack([b_hi, b_lo]),
        "ones": np.ones((2, P), dtype=np.float16),
    }
    total = n_rows * n_cores
    xp = np.zeros((total, D), dtype=np.float32)
    xp[: len(x_rows)] = x_rows
    xh = xp.astype(np.float16)
    xl = (xp - xh.astype(np.float32)).astype(np.float16)
    in_maps = []
    for i in range(n_cores):
        m = dict(base)
        parts = []
        for arr in (xh, xl):
            ac = arr[i * n_rows:(i + 1) * n_rows]
            parts.append(np.ascontiguousarray(
                ac.reshape(nt, P, DT, P).transpose(0, 3, 2, 1)).reshape(nt, P, D))
        m["xt"] = np.concatenate(parts, axis=2)  # [nt, P, 2D]
        in_maps.append(m)
    return in_maps


def _run_exact_f16(x_rows: np.ndarray, centroids: np.ndarray, n_rows: int):
    nc = _cached_nc(("f16x3", n_rows), lambda: build_nc_f16x3(n_rows))
    in_maps = make_f16x3_in_maps(x_rows, centroids, n_rows)
    res = _run_spmd(nc, in_maps, "phase2")
    nt = n_rows // P
    parts = []
    for i in range(NCORES):
        ii = res.results[i]["outi"].reshape(P, nt, 8)
        parts.append(ii.transpose(1, 0, 2).reshape(n_rows, 8)[:, 0])
    out = np.concatenate(parts).astype(np.int32)
    return out[: len(x_rows)], res


def make_f16_in_maps(x: np.ndarray, centroids: np.ndarray,
                     n_rows: int = NSH, n_cores: int = NCORES):
    x = np.ascontiguousarray(x, dtype=np.float32)
    c = np.ascontiguousarray(centroids, dtype=np.float32)
    nt = n_rows // P
    c_norm = (c.astype(np.float64) ** 2).sum(axis=0)
    b = (384.0 - 0.5 * c_norm).astype(np.float32)  # pre-shifted bias
    b_hi = b.astype(np.float16)
    b_lo = (b - b_hi.astype(np.float32)).astype(np.float16)
    base = {
        "c": c.astype(np.float16),
        "bias2": np.stack([b_hi, b_lo]),
        "biasr": np.broadcast_to(b_hi, (P, K)).copy(),
        "ones": np.ones((2, P), dtype=np.float16),
    }
    x16 = x.astype(np.float16)
    in_maps = []
    for i in range(n_cores):
        xc = x16[i * n_rows:(i + 1) * n_rows]
        # xt[t, d, c, col] = x[t*128+col, c*128+d]
        xt = np.ascontiguousarray(
            xc.reshape(nt, P, DT, P).transpose(0, 3, 2, 1)).reshape(nt, P, D)
        m = dict(base)
        m["xt"] = xt
        in_maps.append(m)
    return in_maps


def build_nc(mode: str = "bf16x3", n_rows: int = NSH):
    """Build + compile the per-core Bass program.

    mode: 'bf16x3' (hi/lo split, 3 bf16 passes), 'fp32', 'fp32r', 'bf16'
    """
    assert n_rows % P == 0
    nt = n_rows // P
    nc = bacc.Bacc("TRN2", target_bir_lowering=False, debug=False)

    x = nc.dram_tensor("x", [n_rows, D], F32, kind="ExternalInput").ap()
    bias = nc.dram_tensor("bias", [P, K], F32, kind="ExternalInput").ap()
    out = nc.dram_tensor("out", [n_rows, 1], U32, kind="ExternalOutput").ap()

    split = mode == "bf16x3"
    cdt = BF if mode in ("bf16x3", "bf16") else F32
    mmdt = {"bf16x3": BF, "bf16": BF, "fp32": F32, "fp32r": mybir.dt.float32r}[mode]

    if split:
        c_hi = nc.dram_tensor("c_hi", [D, K], BF, kind="ExternalInput").ap()
        c_lo = nc.dram_tensor("c_lo", [D, K], BF, kind="ExternalInput").ap()
        c_srcs = [c_hi, c_lo]
    else:
        c_full = nc.dram_tensor("c", [D, K], cdt, kind="ExternalInput").ap()
        c_srcs = [c_full]
    ident = nc.dram_tensor("ident", [P, P], mmdt if mmdt != mybir.dt.float32r else F32,
                           kind="ExternalInput").ap()

    with tile.TileContext(nc) as tc, ExitStack() as ctx:
        const = ctx.enter_context(tc.tile_pool(name="const", bufs=1))
        xin_p = ctx.enter_context(tc.tile_pool(name="xin", bufs=3))
        xcast_p = ctx.enter_context(tc.tile_pool(name="xcast", bufs=2))
        xtp_p = ctx.enter_context(tc.tile_pool(name="xtp", bufs=2, space="PSUM"))
        xts_p = ctx.enter_context(tc.tile_pool(name="xts", bufs=2))
        sc_p = ctx.enter_context(tc.tile_pool(name="sc", bufs=1, space="PSUM"))
        ss_p = ctx.enter_context(tc.tile_pool(name="ss", bufs=2))
        mx_p = ctx.enter_context(tc.tile_pool(name="mx", bufs=4))

        # centroids resident in SBUF: [DT][P, K] per source (hi/lo or single)
        c_tiles = []
        for si, csrc in enumerate(c_srcs):
            c3 = csrc.rearrange("(t p) k -> t p k", p=P)
            tiles = []
            for d in range(DT):
                ct = const.tile([P, K], cdt, tag=f"c{si}_{d}")
                nc.sync.dma_start(ct[:], c3[d])
                tiles.append(ct)
            c_tiles.append(tiles)

        bias_t = const.tile([P, K], F32, tag="bias")
        nc.sync.dma_start(bias_t[:], bias[:, :])
        id_t = const.tile([P, P], ident.dtype, tag="ident")
        nc.sync.dma_start(id_t[:], ident[:, :])

        for t in range(nt):
            xin = xin_p.tile([P, D], F32)
            nc.scalar.dma_start(xin[:], x[t * P:(t + 1) * P, :])

            if split:
                xh = xcast_p.tile([P, D], BF, tag="xh")
                nc.scalar.copy(xh[:], xin[:])
                xl = xcast_p.tile([P, D], BF, tag="xl")
                nc.vector.tensor_sub(xl[:], xin[:], xh[:])
                tsrc = [xh, xl]
            elif mode == "bf16":
                xh = xcast_p.tile([P, D], BF, tag="xh")
                nc.scalar.copy(xh[:], xin[:])
                tsrc = [xh]
            else:
                tsrc = [xin]

            # transpose x tiles -> [d, n] layout for matmul weights
            nsrc = len(tsrc)
            tdt = BF if cdt == BF else F32
            xtp = xtp_p.tile([P, D * nsrc], tdt)
            for si, xsrc in enumerate(tsrc):
                for d in range(DT):
                    nc.tensor.transpose(
                        xtp[:, si * D + d * P: si * D + (d + 1) * P],
                        xsrc[:, d * P:(d + 1) * P],
                        id_t[:],
                    )
            xts = xts_p.tile([P, D * nsrc], tdt)
            nc.scalar.copy(xts[:], xtp[:])

            def w(si, d):
                return xts[:, si * D + d * P: si * D + (d + 1) * P]

            banks = []
            for b in range(NB):
                bank_tile = sc_p.tile([P, KW[b]], F32, tag=f"b{b}", name=f"bank{b}")
                banks.append(bank_tile)
            if split:
                # accumulate xh.ch + xh.cl + xl.ch over d
                for d in range(DT):
                    for b in range(NB):
                        nc.tensor.matmul(
                            banks[b][:], w(0, d),
                            c_tiles[0][d][:, KOFF[b]:KOFF[b] + KW[b]],
                            start=(d == 0), stop=False)
                    for b in range(NB):
                        nc.tensor.matmul(
                            banks[b][:], w(0, d),
                            c_tiles[1][d][:, KOFF[b]:KOFF[b] + KW[b]],
                            start=False, stop=False)
                    for b in range(NB):
                        nc.tensor.matmul(
                            banks[b][:], w(1, d),
                            c_tiles[0][d][:, KOFF[b]:KOFF[b] + KW[b]],
                            start=False, stop=(d == DT - 1))
            else:
                for d in range(DT):
                    for b in range(NB):
                        lhs = w(0, d)
                        rhs = c_tiles[0][d][:, KOFF[b]:KOFF[b] + KW[b]]
                        if mode == "fp32r":
                            lhs = lhs.bitcast(mybir.dt.float32r)
                            rhs = rhs.bitcast(mybir.dt.float32r)
                        nc.tensor.matmul(banks[b][:], lhs, rhs,
                                         start=(d == 0), stop=(d == DT - 1))

            ss = ss_p.tile([P, K], F32)
            for b in range(NB):
                nc.vector.tensor_add(
                    ss[:, KOFF[b]:KOFF[b] + KW[b]], banks[b][:],
                    bias_t[:, KOFF[b]:KOFF[b] + KW[b]])

            mxv = mx_p.tile([P, 8], F32, tag="mxv")
            nc.vector.max(mxv[:], ss[:])
            mxi = mx_p.tile([P, 8], U32, tag="mxi")
            nc.vector.max_index(mxi[:], mxv[:], ss[:])
            nc.scalar.dma_start(out[t * P:(t + 1) * P, :], mxi[:, 0:1])

    nc.compile()
    return nc


def make_in_maps(x: np.ndarray, centroids: np.ndarray, mode: str = "bf16x3",
                 n_rows: int = NSH, n_cores: int = NCORES):
    x = np.ascontiguousarray(x, dtype=np.float32)
    c = np.ascontiguousarray(centroids, dtype=np.float32)
    c_norm = (c.astype(np.float64) ** 2).sum(axis=0)
    bias = np.broadcast_to((-0.5 * c_norm).astype(np.float32), (P, K)).copy()

    base = {"bias": bias}
    if mode == "bf16x3":
        c_hi = c.astype(BF16)
        c_lo = (c - c_hi.astype(np.float32)).astype(BF16)
        base["c_hi"] = c_hi
        base["c_lo"] = c_lo
        base["ident"] = np.eye(P, dtype=BF16)
    elif mode == "bf16":
        base["c"] = c.astype(BF16)
        base["ident"] = np.eye(P, dtype=BF16)
    else:
        base["c"] = c
        base["ident"] = np.eye(P, dtype=np.float32)

    in_maps = []
    for i in range(n_cores):
        m = dict(base)
        m["x"] = x[i * n_rows:(i + 1) * n_rows]
        in_maps.append(m)
    return in_maps


_NC_CACHE = {}
LAST_RESULTS = []  # (label, BassKernelResults) of the most recent kernel() call


def _run_spmd(nc, in_maps, label):
    kw = {}
    if os.environ.get("KMEANS_TRACE"):
        import shutil

        kw["trace"] = True
        kw["tmpdir"] = os.environ.get("KMEANS_TRACE_DIR", "/tmp/km_trace") + "_" + label
        shutil.rmtree(kw["tmpdir"], ignore_errors=True)
        os.makedirs(kw["tmpdir"], exist_ok=True)
    res = run_bass_kernel_spmd(nc, in_maps, core_ids=list(range(NCORES)), **kw)
    LAST_RESULTS.append((label, res))
    return res

# Phase-2 capacity: rows per core recomputed exactly. Margin threshold:
# empirical max bf16 score error on randn data is ~0.2; flag anything under
# 4x that. ~5% of rows get flagged at this threshold.
P2_ROWS = 1024
MARGIN_TH = None  # set below after calibration constant


def _cached_nc(key, builder):
    if key not in _NC_CACHE:
        _NC_CACHE[key] = builder()
    return _NC_CACHE[key]


def make_screen_in_maps(x: np.ndarray, centroids: np.ndarray,
                        n_rows: int = NSH, n_cores: int = NCORES):
    x = np.ascontiguousarray(x, dtype=np.float32)
    c = np.ascontiguousarray(centroids, dtype=np.float32)
    c_norm = (c.astype(np.float64) ** 2).sum(axis=0)
    bias = (-0.5 * c_norm).astype(np.float32)
    bias_hi = bias.astype(BF16)
    bias_lo = (bias - bias_hi.astype(np.float32)).astype(BF16)
    base = {
        "c": c.astype(BF16),
        "bias2": np.stack([bias_hi, bias_lo]),
        "ones": np.ones((2, P), dtype=BF16),
        "ident": np.eye(P, dtype=BF16),
    }
    in_maps = []
    for i in range(n_cores):
        m = dict(base)
        m["x"] = x[i * n_rows:(i + 1) * n_rows]
        in_maps.append(m)
    return in_maps


def _run_exact(x_rows: np.ndarray, centroids: np.ndarray, n_rows: int):
    """Run the exact (bf16x3) program on x_rows padded to n_rows*NCORES."""
    nc = _cached_nc(("bf16x3", n_rows), lambda: build_nc("bf16x3", n_rows))
    total = n_rows * NCORES
    xp = np.zeros((total, D), dtype=np.float32)
    xp[: len(x_rows)] = x_rows
    in_maps = make_in_maps(xp, centroids, mode="bf16x3", n_rows=n_rows)
    res = _run_spmd(nc, in_maps, "phase2")
    out = np.concatenate(
        [res.results[i]["out"].reshape(n_rows) for i in range(NCORES)])
    return out[: len(x_rows)], res


def kernel(x: np.ndarray, centroids: np.ndarray) -> np.ndarray:
    mode = os.environ.get("KMEANS_MODE", "f16")
    LAST_RESULTS.clear()
    x = np.asarray(x)
    centroids = np.asarray(centroids)

    if mode == "f16":
        # phase 1: fp16 screen with top-8 values + indices
        nc1 = _cached_nc(("f16", NSH), lambda: build_nc_f16(NSH))
        in_maps = make_f16_in_maps(x, centroids)
        res1 = _run_spmd(nc1, in_maps, "phase1")
        nt = NSH // P
        idx_parts, val_parts = [], []
        for i in range(NCORES):
            vi = res1.results[i]["outv"].reshape(P, nt, 8).astype(np.float32)
            ii = res1.results[i]["outi"].reshape(P, nt, 8)
            # row n = t*128 + p  ->  [t, p]
            val_parts.append(vi.transpose(1, 0, 2).reshape(NSH, 8))
            idx_parts.append(ii.transpose(1, 0, 2).reshape(NSH, 8))
        vals = np.concatenate(val_parts)
        idx = np.concatenate(idx_parts)[:, 0].astype(np.int32)

        # fp16 values: quantization ~0.06-0.125 near |s'|<=256 plus fp16
        # matmul error ~0.05 -> th=0.3 flags everything at risk
        margin = vals[:, 0] - vals[:, 1]
        th = float(os.environ.get("KMEANS_MARGIN_TH", "0.3"))
        flagged = np.flatnonzero(margin < th)

        # cap at 4096 (phase-2's 512-rows/core tier): if slightly more are
        # flagged, drop the widest margins -- those are far above the fp16
        # screen's worst-case error, so they cannot actually be mis-picks
        CAP = 4096
        if len(flagged) > CAP:
            keep = np.argpartition(margin[flagged], CAP - 1)[:CAP]
            flagged = flagged[keep]
        if os.environ.get("KMEANS_DEBUG"):
            print(f"[f16] flagged={len(flagged)} ({100 * len(flagged) / N:.2f}%)",
                  flush=True)
        sizes = [128, 256, 512, 768, 1024, 2048]
        per_core = min((s for s in sizes if s * NCORES >= len(flagged)),
                       default=sizes[-1])
        cap = per_core * NCORES
        for s in range(0, len(flagged), cap):
            rows = flagged[s:s + cap]
            exact_idx, _ = _run_exact_f16(x[rows], centroids, per_core)
            idx[rows] = exact_idx
        return idx

    if mode != "hybrid":
        nc = _cached_nc((mode, NSH), lambda: build_nc(mode=mode))
        in_maps = make_in_maps(x, centroids, mode=mode)
        res = _run_spmd(nc, in_maps, mode)
        parts = [res.results[i]["out"].reshape(NSH) for i in range(NCORES)]
        return np.concatenate(parts).astype(np.int32)

    # phase 1: bf16 screen with top-2 margins
    nc1 = _cached_nc(("screen", NSH), lambda: build_nc_screen(NSH))
    in_maps = make_screen_in_maps(x, centroids)
    res1 = _run_spmd(nc1, in_maps, "phase1")
    idx = np.concatenate(
        [res1.results[i]["out"].reshape(NSH) for i in range(NCORES)]
    ).astype(np.int32)
    vals = np.concatenate(
        [res1.results[i]["vals"].reshape(NSH, 2) for i in range(NCORES)])

    margin = vals[:, 0] - vals[:, 1]
    th = float(os.environ.get("KMEANS_MARGIN_TH", "0.8"))
    flagged = np.flatnonzero(margin < th)

    # phase 2: exact recompute of flagged rows; pick the smallest padded
    # program that covers the count, chunking in the (unexpected) overflow case
    sizes = [512, 1024, 1536, 2048]
    per_core = min((s for s in sizes if s * NCORES >= len(flagged)),
                   default=sizes[-1])
    cap = per_core * NCORES
    for s in range(0, len(flagged), cap):
        rows = flagged[s:s + cap]
        exact_idx, _ = _run_exact(x[rows], centroids, per_core)
        idx[rows] = exact_idx
    return idx

